# revision 6
# baseline (speedup 1.0000x reference)
"""GAT (2-layer, 8-head then 1-head) on 8 Trainium2 NeuronCores. v2.

Design: dst-shard nodes across 8 cores. Per layer, every core holds a bf16
node-feature table shard [NPAD, 128] (256B-pitch rows, only the leading
cols used: L1 row = [h(64)|al_hi(8)|al_lo(8)], L2 row = [y2(40)|hi|lo]),
AllGathered to all cores. Edges (dst-owned) are streamed in (block,
shard-PAIR, window)-aligned order with core-common structure; rows are
fetched with dma_gather using int16 idx into the 25088-row pair sub-table,
gathering only 160B (L1) / 96B (L2) of each 256B-pitch row. Per 128-edge
tile, one-hot S (edges x window-nodes, built via 4x tensor_scalar is_equal)
aggregates messages+exp on the PE into per-window PSUM; one-hot S^T looks
up al_dst per edge via PE. W2 is pre-applied in the L1 epilogue (linearity)
so L2 aggregates 40-wide. Softmax without max-subtraction (logits O(4));
log_softmax's Ln is batched at the end so only one act-table switch occurs.
"""
import sys
import numpy as np

sys.path.insert(0, "/opt/trn_rl_repo")
import ml_dtypes

BF = ml_dtypes.bfloat16

N = 100000
F_IN = 128
HID = 8
HEADS = 8
CLASSES = 40
NEG = 0.2
NC = 8

FULL_CFG = dict(
    ncores=8, nshard=12500, npad=12544, wb=4, ni_max=3072, f_in=128,
    heads=8, hid=8, classes=40, neg=0.2, nsp=4, ggc1=128, ggc2=128,
)

_PATCHED = [False]


def _patch_dma_gather():
    """Relax dma_gather's elem_size%256 assert (row PITCH stays 256B)."""
    if _PATCHED[0]:
        return
    import inspect
    import concourse.bass as bass

    src = inspect.getsource(bass.BassGpSimd.dma_gather)
    needle = "elem_size_bytes > 0 and elem_size_bytes % 256 == 0"
    assert needle in src, "dma_gather source changed; update patch"
    src = src.replace(needle, "elem_size_bytes > 0")
    src = "\n".join(line[4:] for line in src.split("\n"))
    ns = vars(bass).copy()
    exec(compile(src, "<patched_dma_gather>", "exec"), ns)
    bass.BassGpSimd.dma_gather = ns["dma_gather"]
    _PATCHED[0] = True


def _ceil(a, b):
    return -(-a // b)


# ---------------------------------------------------------------- host prep
def prep_structure(src, dst, cfg):
    """Build core-common call/tile/matmul structure + per-core arrays.

    Edges are grouped by (dst-core, block-of-4-windows, src shard-PAIR,
    window); group sizes are padded to the max over cores so the program is
    identical on all cores. Returns (st, percore).
    """
    nc_, nsh, npad, wb = cfg["ncores"], cfg["nshard"], cfg["npad"], cfg["wb"]
    nsp = cfg["nsp"]
    per_pair = 2 * nsh                     # 25000 global nodes per pair
    nwin = npad // 128
    nblk = _ceil(nwin, wb)
    NI = cfg["ni_max"]
    NT = NI // 128

    core = dst // nsh
    dstl = dst % nsh
    spair = src // per_pair
    # local row within the pair sub-table [2*npad rows]
    sl_in_pair = src % per_pair
    slocal = np.where(sl_in_pair < nsh, sl_in_pair,
                      sl_in_pair - nsh + npad)
    win = dstl // 128
    blk = win // wb

    nwb = [min(wb, nwin - b * wb) for b in range(nblk)]
    counts = np.zeros((nc_, nblk, nsp, nwin), dtype=np.int64)
    np.add.at(counts, (core, blk, spair, win), 1)
    common = counts.max(axis=0)            # [nblk, nsp, nwin]

    percore_edges = []
    for c in range(nc_):
        m = core == c
        key = (blk[m].astype(np.int64) * nsp + spair[m]) * nwin + win[m]
        o = np.argsort(key, kind="stable")
        percore_edges.append((key[o], slocal[m][o], dstl[m][o], win[m][o]))

    PAD_ROW = nsh          # pad row inside pair's first shard (pneg als)
    calls = []
    pc_idx = [[] for _ in range(nc_)]
    pc_dcol = [[] for _ in range(nc_)]
    pc_drow = [[] for _ in range(nc_)]
    win_mms = {}
    for b in range(nblk):
        for s in range(nsp):
            streams = []
            for c in range(nc_):
                kk, sl, dl, wn = percore_edges[c]
                segs = []
                for w in range(b * wb, b * wb + nwb[b]):
                    kval = (b * nsp + s) * nwin + w
                    lo = np.searchsorted(kk, kval, "left")
                    hi = np.searchsorted(kk, kval, "right")
                    n_common = common[b, s, w]
                    seg_s = np.full(n_common, PAD_ROW, dtype=np.int64)
                    seg_w = np.full(n_common, w, dtype=np.int64)
                    seg_s[: hi - lo] = sl[lo:hi]
                    seg_w[: hi - lo] = wn[lo:hi]
                    seg_d = np.full(n_common, -1, dtype=np.int64)
                    seg_d[: hi - lo] = dl[lo:hi]
                    segs.append(np.stack([seg_s, seg_w, seg_d]))
                streams.append(np.concatenate(segs, axis=1))
            L = streams[0].shape[1]
            pos = 0
            while pos < L:
                ni_real = min(NI, L - pos)
                ni = _ceil(ni_real, 128) * 128
                ntile = ni // 128
                wseg = streams[0][1][pos:pos + ni_real]
                tiles = []
                for t in range(ntile):
                    a, z = t * 128, min((t + 1) * 128, ni_real)
                    if a < ni_real:
                        tw = wseg[a:z]
                        w1 = int(tw.min())
                        wmax = int(tw.max())
                        assert wmax - w1 <= 1, "tile spans >2 windows"
                        straddle = wmax > w1
                    else:
                        w1, straddle = int(wseg[-1]), False
                    tiles.append((w1, straddle))
                cid = len(calls)
                mms = []
                for t, (w1, straddle) in enumerate(tiles):
                    ks = [0, 1] if straddle else [0]
                    for k in ks:
                        w = w1 + k
                        mm_id = (cid, t, k, w)
                        win_mms.setdefault((b, w), []).append(mm_id)
                        mms.append(mm_id)
                calls.append(dict(kind="stream", blk=b, shard=s, ni=ni,
                                  ntile=ntile, tiles=tiles, mms=mms))
                for c in range(nc_):
                    ss, ww, dd = streams[c]
                    sl_call = np.full(ni, PAD_ROW, dtype=np.int64)
                    rel_call = np.full(ni, 300.0, dtype=np.float64)
                    nreal = min(ni_real, L - pos)
                    sl_call[:nreal] = ss[pos:pos + nreal]
                    for t in range(ntile):
                        a, z = t * 128, min((t + 1) * 128, nreal)
                        if a >= nreal:
                            break
                        w1 = tiles[t][0]
                        dv = dd[a:z]
                        wv = ww[a:z]
                        rel = (wv - w1) * 128 + (dv - wv * 128)
                        rel = np.where(dv < 0, 300.0, rel)
                        rel_call[a:z] = rel
                    iw = sl_call.reshape(ni // 16, 16).T.astype(np.int16)
                    pc_idx[c].append(np.tile(iw, (8, 1)))
                    pc_dcol[c].append(
                        rel_call.reshape(ntile, 128).T.astype(np.float32))
                    pc_drow[c].append(rel_call.astype(BF))
                pos += ni_real
        for w in range(b * wb, b * wb + nwb[b]):
            cid = len(calls)
            mm_id = (cid, 0, 0, w)
            win_mms.setdefault((b, w), []).append(mm_id)
            calls.append(dict(kind="self", blk=b, w=w, mms=[mm_id]))

    startset, stopset = set(), set()
    for (b, w), ms in win_mms.items():
        startset.add(ms[0])
        stopset.add(ms[-1])
    for cl in calls:
        cl["flags"] = [(m, m in startset, m in stopset) for m in cl["mms"]]

    ncalls = len(calls)
    idx_t = [np.zeros((128, (NI // 16) * ncalls), np.int16) for _ in range(nc_)]
    dcol_t = [np.zeros((128, NT * ncalls), np.float32) for _ in range(nc_)]
    drow_t = [np.full((1, NI * ncalls), 300.0, BF) for _ in range(nc_)]
    for c in range(nc_):
        j = 0
        for i, cl in enumerate(calls):
            if cl["kind"] == "self":
                continue
            ni, nt = cl["ni"], cl["ntile"]
            idx_t[c][:, i * (NI // 16): i * (NI // 16) + ni // 16] = pc_idx[c][j]
            dcol_t[c][:, i * NT: i * NT + nt] = pc_dcol[c][j]
            drow_t[c][0, i * NI: i * NI + ni] = pc_drow[c][j]
            j += 1

    st = dict(calls=calls, nwin=nwin, nblk=nblk, nwb=nwb, ncalls=ncalls,
              win_mms=win_mms)
    percore = [dict(idx=idx_t[c], dcol=dcol_t[c], drow=drow_t[c])
               for c in range(nc_)]
    return st, percore


# ---------------------------------------------------------------- program
def build_nc(cfg, st):
    import concourse.bass as bass
    import concourse.bacc as bacc
    import concourse.tile as tile
    import concourse.mybir as mybir
    from concourse.masks import make_identity

    _patch_dma_gather()

    bf16, f32 = mybir.dt.bfloat16, mybir.dt.float32
    i16, i32 = mybir.dt.int16, mybir.dt.int32
    AL = mybir.AluOpType
    AF = mybir.ActivationFunctionType
    ax_x = mybir.AxisListType.X

    nc_, nsh, npad = cfg["ncores"], cfg["nshard"], cfg["npad"]
    nsp = cfg["nsp"]
    H, C1, CL = cfg["heads"], cfg["hid"], cfg["classes"]
    D1 = H * C1                      # 64
    NEGS = cfg["neg"]
    NI = cfg["ni_max"]
    NT = NI // 128
    nwin, nblk, nwb = st["nwin"], st["nblk"], st["nwb"]
    ncalls = st["ncalls"]
    NTOT = nc_ * npad
    ntile_x = npad // 128

    GC1 = D1 + 2 * H                 # 80: [h64 | hi8 | lo8]
    GC2 = CL + 2                     # 42: [y2 40 | hi | lo] (gather 48)
    GC2P = 48
    RH1 = D1 + H                     # 72
    RH2 = CL + 1                     # 41
    W1C = D1 + 2 * H                 # producer matmul width (80)

    nc = bacc.Bacc("TRN2", target_bir_lowering=False, debug=False,
                   enable_asserts=False, num_devices=nc_, num_swdge_queues=4)

    # ---- I/O
    x_T = nc.dram_tensor("x_T", [cfg["f_in"], npad], f32, kind="ExternalInput")
    w1cat = nc.dram_tensor("w1cat", [cfg["f_in"], W1C], f32,
                           kind="ExternalInput")
    b1row = nc.dram_tensor("b1row", [1, D1], f32, kind="ExternalInput")
    a2srow = nc.dram_tensor("a2srow", [1, CL], f32, kind="ExternalInput")
    a2drow = nc.dram_tensor("a2drow", [1, CL], f32, kind="ExternalInput")
    w2b = nc.dram_tensor("w2b", [D1, CL], bf16, kind="ExternalInput")
    b2row = nc.dram_tensor("b2row", [1, CL], f32, kind="ExternalInput")
    idx_in = nc.dram_tensor("idx_in", [128, (NI // 16) * ncalls], i16,
                            kind="ExternalInput")
    dcol_in = nc.dram_tensor("dcol_in", [128, NT * ncalls], f32,
                             kind="ExternalInput")
    drow_in = nc.dram_tensor("drow_in", [1, NI * ncalls], bf16,
                             kind="ExternalInput")
    pmask_in = nc.dram_tensor("pmask", [128, 1], f32, kind="ExternalInput")
    pneg_in = nc.dram_tensor("pneg", [128, 1], f32, kind="ExternalInput")
    out_d = nc.dram_tensor("out", [npad, CL], f32, kind="ExternalOutput")

    with tile.TileContext(nc) as tc:
        with (
            tc.tile_pool(name="const", bufs=1) as cpool,
            tc.tile_pool(name="sb", bufs=3) as sb,
            tc.tile_pool(name="gpool", bufs=4) as gp,
            tc.tile_pool(name="spool", bufs=3) as sp,
            tc.tile_pool(name="meta", bufs=4) as mp,
            tc.tile_pool(name="epi", bufs=2) as ep,
            tc.tile_pool(name="res", bufs=1) as rp,
            tc.tile_pool(name="pwin", bufs=max(nwb) + 1, space="PSUM") as pw,
            tc.tile_pool(name="pald", bufs=1, space="PSUM") as pa,
            tc.tile_pool(name="pma", bufs=1, space="PSUM") as pm,
            tc.tile_pool(name="pmb", bufs=1, space="PSUM") as pmb,
            tc.tile_pool(name="dram", bufs=1, space="DRAM") as dp,
        ):
            # ---------- constants
            ident = cpool.tile([128, 128], f32)
            make_identity(nc, ident[:])
            identb = cpool.tile([128, 128], bf16)
            nc.vector.tensor_copy(identb[:], ident[:])
            iota_i = cpool.tile([128, 128], i32)
            nc.gpsimd.iota(iota_i[:], pattern=[[1, 128]], base=0,
                           channel_multiplier=0)
            iota_mat = cpool.tile([128, 128], bf16)
            nc.vector.tensor_copy(iota_mat[:], iota_i[:])
            iota_mat2 = cpool.tile([128, 128], bf16)
            nc.vector.tensor_scalar_add(iota_mat2[:], iota_mat[:], 128.0)
            ic_i = cpool.tile([128, 1], i32)
            nc.gpsimd.iota(ic_i[:], pattern=[[0, 1]], base=0,
                           channel_multiplier=1)
            iota_col = cpool.tile([128, 1], f32)
            nc.vector.tensor_copy(iota_col[:], ic_i[:])
            iota_col2 = cpool.tile([128, 1], f32)
            nc.vector.tensor_scalar_add(iota_col2[:], iota_col[:], 128.0)
            b1m = cpool.tile([128, D1], f32)
            nc.sync.dma_start(out=b1m[:], in_=b1row[:].to_broadcast([128, D1]))
            a2sm = cpool.tile([128, CL], f32)
            nc.sync.dma_start(out=a2sm[:], in_=a2srow[:].to_broadcast([128, CL]))
            a2dm = cpool.tile([128, CL], f32)
            nc.sync.dma_start(out=a2dm[:], in_=a2drow[:].to_broadcast([128, CL]))
            b2m = cpool.tile([128, CL], f32)
            nc.sync.dma_start(out=b2m[:], in_=b2row[:].to_broadcast([128, CL]))
            w1c_sb = cpool.tile([cfg["f_in"], W1C], f32)
            nc.sync.dma_start(out=w1c_sb[:], in_=w1cat[:])
            w2b_sb = cpool.tile([D1, CL], bf16)
            nc.sync.dma_start(out=w2b_sb[:], in_=w2b[:])
            pmask = cpool.tile([128, 1], f32)
            nc.sync.dma_start(out=pmask[:], in_=pmask_in[:])
            pneg = cpool.tile([128, 1], f32)
            nc.sync.dma_start(out=pneg[:], in_=pneg_in[:])
            zcol = cpool.tile([128, 1], f32)
            nc.vector.memset(zcol[:], 0.0)

            # resident tables
            al1w = rp.tile([128, 2 * H * nwin], bf16)     # [hi8|lo8] per win
            al2w = rp.tile([128, 2 * nwin], bf16)         # [hi|lo] per win
            lgs = rp.tile([128, CL * nwin], f32)          # logits - max
            sms = rp.tile([128, nwin], f32)               # sum(exp)

            # DRAM tables (256B-pitch rows; only leading cols used)
            t1_own = dp.tile([npad, 128], bf16)
            t1_full = dp.tile([NTOT, 128], bf16)
            t2_own = dp.tile([npad, 128], bf16)
            t2_full = dp.tile([NTOT, 128], bf16)

            # ---------------- P0: produce T1 + al1 window tables
            for t in range(ntile_x):
                xt = sb.tile([cfg["f_in"], 128], f32, tag="xt")
                nc.sync.dma_start(out=xt[:], in_=x_T[:, t * 128:(t + 1) * 128])
                ps = pm.tile([128, W1C], f32, space="PSUM", tag="pm")
                nc.tensor.matmul(ps[:], lhsT=xt[:], rhs=w1c_sb[:],
                                 start=True, stop=True)
                t1sb = sb.tile([128, GC1], bf16, tag="t1sb")
                nc.vector.tensor_copy(t1sb[:, 0:D1], ps[:, 0:D1])
                nc.vector.tensor_copy(t1sb[:, D1:D1 + H], ps[:, D1:D1 + H])
                nc.vector.tensor_tensor(out=t1sb[:, D1 + H:D1 + 2 * H],
                                        in0=ps[:, D1:D1 + H],
                                        in1=t1sb[:, D1:D1 + H],
                                        op=AL.subtract)
                o = 2 * H * t
                nc.vector.tensor_copy(al1w[:, o:o + H], ps[:, D1 + H:W1C])
                nc.vector.tensor_tensor(out=al1w[:, o + H:o + 2 * H],
                                        in0=ps[:, D1 + H:W1C],
                                        in1=al1w[:, o:o + H], op=AL.subtract)
                if t == ntile_x - 1 and npad > nsh:
                    nc.vector.scalar_tensor_tensor(
                        out=t1sb[:, 0:D1], in0=t1sb[:, 0:D1], scalar=pmask[:],
                        in1=zcol[:].to_broadcast([128, D1]),
                        op0=AL.mult, op1=AL.add)
                    nc.vector.scalar_tensor_tensor(
                        out=t1sb[:, D1:D1 + H], in0=t1sb[:, D1:D1 + H],
                        scalar=pmask[:], in1=pneg[:].to_broadcast([128, H]),
                        op0=AL.mult, op1=AL.add)
                    nc.vector.scalar_tensor_tensor(
                        out=t1sb[:, D1 + H:D1 + 2 * H],
                        in0=t1sb[:, D1 + H:D1 + 2 * H],
                        scalar=pmask[:], in1=zcol[:].to_broadcast([128, H]),
                        op0=AL.mult, op1=AL.add)
                nc.sync.dma_start(out=t1_own[t * 128:(t + 1) * 128, 0:GC1],
                                  in_=t1sb[:])

            nc.gpsimd.collective_compute(
                "AllGather", AL.bypass,
                replica_groups=[list(range(nc_))],
                ins=[t1_own.opt()], outs=[t1_full.opt()],
            )

            # ---------------- shared edge pass
            def edge_pass(tfull, town, alw, nal, mc, gc, gcp, rhw):
                """nal: attn scalars/edge; mc: msg cols; gc: used row cols;
                gcp: gathered cols; rhw: rhs width = mc + nal."""
                blk_psums = {}
                cph = mc // nal
                for ci, cl in enumerate(st["calls"]):
                    if cl["kind"] == "self":
                        w = cl["w"]
                        gs = gp.tile([128, gcp], bf16, tag="gs")
                        nc.sync.dma_start(
                            out=gs[:],
                            in_=town[w * 128:(w + 1) * 128, 0:gcp])
                        es = sb.tile([128, nal], f32, tag="es")
                        nc.vector.tensor_tensor(
                            out=es[:], in0=gs[:, mc:mc + nal],
                            in1=gs[:, mc + nal:mc + 2 * nal], op=AL.add)
                        ed = sb.tile([128, nal], f32, tag="ed")
                        nc.vector.tensor_tensor(
                            out=ed[:], in0=alw[:, 2 * nal * w:2 * nal * w + nal],
                            in1=alw[:, 2 * nal * w + nal:2 * nal * (w + 1)],
                            op=AL.add)
                        nc.vector.tensor_tensor(out=es[:], in0=es[:], in1=ed[:],
                                                op=AL.add)
                        nc.vector.scalar_tensor_tensor(
                            out=es[:], in0=es[:], scalar=NEGS, in1=es[:],
                            op0=AL.mult, op1=AL.max)
                        rhs_s = sb.tile([128, RH1], bf16, tag="rhss")
                        nc.scalar.activation(rhs_s[:, mc:mc + nal], es[:],
                                             AF.Exp)
                        nc.vector.tensor_tensor(
                            out=rhs_s[:, 0:mc].rearrange("p (a c) -> p a c",
                                                         c=cph),
                            in0=gs[:, 0:mc].rearrange("p (a c) -> p a c",
                                                      c=cph),
                            in1=rhs_s[:, mc:mc + nal]
                            .broadcast_to([128, nal, cph]),
                            op=AL.mult)
                        (mm, fstart, fstop) = cl["flags"][0]
                        key = (cl["blk"], w)
                        pt = blk_psums.get(key)
                        if pt is None:
                            pt = pw.tile([128, RH1], f32, space="PSUM",
                                         tag="pwin")
                            blk_psums[key] = pt
                        nc.tensor.matmul(pt[:, 0:rhw], lhsT=identb[:],
                                         rhs=rhs_s[:, 0:rhw],
                                         start=fstart, stop=fstop)
                        if fstop:
                            yield w, blk_psums.pop(key)
                        continue
                    b, s_, ni, nt = cl["blk"], cl["shard"], cl["ni"], cl["ntile"]
                    dcol = mp.tile([128, NT], f32, tag="dcol")
                    nc.sync.dma_start(out=dcol[:, 0:nt],
                                      in_=dcol_in[:, ci * NT:ci * NT + nt])
                    drep = mp.tile([128, NI], bf16, tag="drep")
                    nc.sync.dma_start(
                        out=drep[:, 0:ni],
                        in_=drow_in[:, ci * NI:ci * NI + ni]
                        .to_broadcast([128, ni]))
                    idxt = mp.tile([128, NI // 16], i16, tag="idxt")
                    nc.sync.dma_start(
                        out=idxt[:, 0:ni // 16],
                        in_=idx_in[:, ci * (NI // 16):ci * (NI // 16) + ni // 16])
                    # gather (gcp cols of each 256B-pitch row); the SWDGE
                    # ucode caps num_idxs at 1024, so issue sub-gathers on
                    # rotating queues
                    g = gp.tile([128, NT * gcp], bf16, tag="g")
                    qn = [0]
                    for a in range(0, ni, 1024):
                        z = min(a + 1024, ni)
                        nc.gpsimd.dma_gather(
                            g[:, (a // 128) * gcp:(z // 128) * gcp]
                            .rearrange("p (b e) -> p b e", e=gcp),
                            tfull[s_ * 2 * npad:(s_ + 1) * 2 * npad, 0:gcp],
                            idxt[:, a // 16:z // 16], z - a, z - a, gcp,
                            elem_step=128, single_packet=True,
                            queue_num=(ci + qn[0]) % 4)
                        qn[0] += 1
                    # one-hot S builds (4x tensor_scalar)
                    s1 = sp.tile([128, NI], bf16, tag="s1")
                    any_straddle = any(x[1] for x in cl["tiles"])
                    if any_straddle:
                        s2 = sp.tile([128, NI], bf16, tag="s2")
                    for t, (w1, straddle) in enumerate(cl["tiles"]):
                        nc.vector.tensor_scalar(
                            out=s1[:, t * 128:(t + 1) * 128],
                            in0=iota_mat[:], scalar1=dcol[:, t:t + 1],
                            scalar2=None, op0=AL.is_equal)
                        if straddle:
                            nc.vector.tensor_scalar(
                                out=s2[:, t * 128:(t + 1) * 128],
                                in0=iota_mat2[:], scalar1=dcol[:, t:t + 1],
                                scalar2=None, op0=AL.is_equal)
                    st1 = sp.tile([128, NI], bf16, tag="st1")
                    nc.vector.tensor_scalar(
                        out=st1[:, 0:ni], in0=drep[:, 0:ni],
                        scalar1=iota_col[:], scalar2=None, op0=AL.is_equal)
                    if any_straddle:
                        st2 = sp.tile([128, NI], bf16, tag="st2")
                        nc.vector.tensor_scalar(
                            out=st2[:, 0:ni], in0=drep[:, 0:ni],
                            scalar1=iota_col2[:], scalar2=None,
                            op0=AL.is_equal)
                    # al_dst lookup per tile -> psum [128, nt*nal];
                    # hi and lo parts accumulate on the PE (start/stop chain)
                    pald = pa.tile([128, NT * nal], f32, space="PSUM",
                                   tag="pald")
                    for t, (w1, straddle) in enumerate(cl["tiles"]):
                        ks = [0, 1] if straddle else [0]
                        mmparts = [(k, part) for k in ks for part in (0, 1)]
                        for i, (k, part) in enumerate(mmparts):
                            w = w1 + k
                            stm = st1 if k == 0 else st2
                            o = 2 * nal * w + part * nal
                            nc.tensor.matmul(
                                pald[:, t * nal:(t + 1) * nal],
                                lhsT=stm[:, t * 128:(t + 1) * 128],
                                rhs=alw[:, o:o + nal],
                                start=(i == 0), stop=(i == len(mmparts) - 1))
                    # e = (als_hi+als_lo) + (ald_hi+ald_lo); leaky
                    eals = sb.tile([128, NT * nal], f32, tag="eals")
                    nc.vector.tensor_tensor(
                        out=eals[:, 0:nt * nal]
                        .rearrange("p (b a) -> p b a", a=nal),
                        in0=g[:, 0:nt * gcp].rearrange("p (b e) -> p b e",
                                                       e=gcp)
                        [:, :, mc:mc + nal],
                        in1=g[:, 0:nt * gcp].rearrange("p (b e) -> p b e",
                                                       e=gcp)
                        [:, :, mc + nal:mc + 2 * nal],
                        op=AL.add)
                    ee = sb.tile([128, NT * nal], f32, tag="ee")
                    nc.vector.tensor_tensor(out=ee[:, 0:nt * nal],
                                            in0=eals[:, 0:nt * nal],
                                            in1=pald[:, 0:nt * nal], op=AL.add)
                    nc.vector.scalar_tensor_tensor(
                        out=ee[:, 0:nt * nal], in0=ee[:, 0:nt * nal],
                        scalar=NEGS, in1=ee[:, 0:nt * nal],
                        op0=AL.mult, op1=AL.max)
                    # rhs assembly
                    rhs = sb.tile([128, NT * rhw], bf16, tag="rhs")
                    nc.scalar.activation(
                        rhs[:, 0:nt * rhw].rearrange("p (b r) -> p b r", r=rhw)
                        [:, :, mc:mc + nal],
                        ee[:, 0:nt * nal].rearrange("p (b a) -> p b a", a=nal),
                        AF.Exp)
                    nc.vector.tensor_tensor(
                        out=rhs[:, 0:nt * rhw]
                        .rearrange("p (b r) -> p b r", r=rhw)[:, :, 0:mc]
                        .rearrange("p b (a c) -> p b a c", c=cph),
                        in0=g[:, 0:nt * gcp].rearrange("p (b e) -> p b e",
                                                       e=gcp)
                        [:, :, 0:mc].rearrange("p b (a c) -> p b a c", c=cph),
                        in1=rhs[:, 0:nt * rhw]
                        .rearrange("p (b r) -> p b r", r=rhw)
                        [:, :, mc:mc + nal]
                        .broadcast_to([128, nt, nal, cph]),
                        op=AL.mult)
                    # aggregation matmuls
                    for (mm, fstart, fstop) in cl["flags"]:
                        _, t, k, w = mm
                        smat = s1 if k == 0 else s2
                        key = (cl["blk"], w)
                        pt = blk_psums.get(key)
                        if pt is None:
                            pt = pw.tile([128, RH1], f32, space="PSUM",
                                         tag="pwin")
                            blk_psums[key] = pt
                        nc.tensor.matmul(
                            pt[:, 0:rhw],
                            lhsT=smat[:, t * 128:(t + 1) * 128],
                            rhs=rhs[:, t * rhw:(t + 1) * rhw],
                            start=fstart, stop=fstop)
                    for (mm, fstart, fstop) in cl["flags"]:
                        if not fstop:
                            continue
                        _, t, k, w = mm
                        key = (cl["blk"], w)
                        yield w, blk_psums.pop(key)

            # ---------------- L1 pass + epilogue -> T2 (W2 pre-applied)
            for w, pt in edge_pass(t1_full, t1_own, al1w, H, D1, GC1,
                                   cfg["ggc1"], RH1):
                rc = ep.tile([128, H], f32, tag="rc1")
                nc.vector.reciprocal(rc[:], pt[:, D1:D1 + H])
                nc.vector.tensor_scalar_min(rc[:], rc[:], 1e30)
                o1 = ep.tile([128, D1], f32, tag="o1")
                nc.vector.tensor_tensor(
                    out=o1[:].rearrange("p (h c) -> p h c", c=C1),
                    in0=pt[:, 0:D1].rearrange("p (h c) -> p h c", c=C1),
                    in1=rc[:].broadcast_to([128, H, C1]),
                    op=AL.mult)
                nc.vector.tensor_tensor(out=o1[:], in0=o1[:], in1=b1m[:],
                                        op=AL.add)
                r1 = ep.tile([128, D1], f32, tag="r1")
                nc.scalar.activation(r1[:], o1[:], AF.Relu)
                # y2 = relu1 @ W2 via transpose + matmul
                trp = pmb.tile([D1, 128], f32, space="PSUM", tag="trp")
                nc.tensor.transpose(out=trp[:], in_=r1[:], identity=ident[:])
                trs = ep.tile([D1, 128], bf16, tag="trs")
                nc.vector.tensor_copy(trs[:], trp[:])
                y2p = pm.tile([128, CL], f32, space="PSUM", tag="pm")
                nc.tensor.matmul(y2p[:], lhsT=trs[:], rhs=w2b_sb[:],
                                 start=True, stop=True)
                t2sb = ep.tile([128, GC2P], bf16, tag="t2sb")
                nc.vector.tensor_copy(t2sb[:, 0:CL], y2p[:])
                tmp = ep.tile([128, CL], f32, tag="altmp")
                a2s = ep.tile([128, 1], f32, tag="a2s")
                nc.vector.tensor_tensor(out=tmp[:], in0=y2p[:], in1=a2sm[:],
                                        op=AL.mult)
                nc.vector.tensor_reduce(a2s[:], tmp[:], axis=ax_x, op=AL.add)
                a2d = ep.tile([128, 1], f32, tag="a2d")
                nc.vector.tensor_tensor(out=tmp[:], in0=y2p[:], in1=a2dm[:],
                                        op=AL.mult)
                nc.vector.tensor_reduce(a2d[:], tmp[:], axis=ax_x, op=AL.add)
                nc.vector.tensor_copy(t2sb[:, CL:CL + 1], a2s[:])
                nc.vector.tensor_tensor(out=t2sb[:, CL + 1:CL + 2],
                                        in0=a2s[:], in1=t2sb[:, CL:CL + 1],
                                        op=AL.subtract)
                nc.vector.memset(t2sb[:, CL + 2:GC2P], 0.0)
                nc.vector.tensor_copy(al2w[:, 2 * w:2 * w + 1], a2d[:])
                nc.vector.tensor_tensor(out=al2w[:, 2 * w + 1:2 * w + 2],
                                        in0=a2d[:], in1=al2w[:, 2 * w:2 * w + 1],
                                        op=AL.subtract)
                if w == nwin - 1 and npad > nsh:
                    nc.vector.scalar_tensor_tensor(
                        out=t2sb[:, 0:CL], in0=t2sb[:, 0:CL], scalar=pmask[:],
                        in1=zcol[:].to_broadcast([128, CL]),
                        op0=AL.mult, op1=AL.add)
                    nc.vector.scalar_tensor_tensor(
                        out=t2sb[:, CL:CL + 1], in0=t2sb[:, CL:CL + 1],
                        scalar=pmask[:], in1=pneg[:], op0=AL.mult, op1=AL.add)
                    nc.vector.scalar_tensor_tensor(
                        out=t2sb[:, CL + 1:CL + 2], in0=t2sb[:, CL + 1:CL + 2],
                        scalar=pmask[:], in1=zcol[:], op0=AL.mult, op1=AL.add)
                    nc.vector.scalar_tensor_tensor(
                        out=al2w[:, 2 * w:2 * w + 2],
                        in0=al2w[:, 2 * w:2 * w + 2],
                        scalar=pmask[:], in1=zcol[:].to_broadcast([128, 2]),
                        op0=AL.mult, op1=AL.add)
                nc.sync.dma_start(out=t2_own[w * 128:(w + 1) * 128, 0:GC2P],
                                  in_=t2sb[:])

            nc.gpsimd.collective_compute(
                "AllGather", AL.bypass,
                replica_groups=[list(range(nc_))],
                ins=[t2_own.opt()], outs=[t2_full.opt()],
            )

            # ---------------- L2 pass + epilogue -> resident logits
            for w, pt in edge_pass(t2_full, t2_own, al2w, 1, CL, GC2,
                                   cfg["ggc2"], RH2):
                rc = ep.tile([128, 1], f32, tag="rc2")
                nc.vector.reciprocal(rc[:], pt[:, CL:CL + 1])
                nc.vector.tensor_scalar_min(rc[:], rc[:], 1e30)
                lg = ep.tile([128, CL], f32, tag="lg")
                nc.vector.tensor_tensor(
                    out=lg[:], in0=pt[:, 0:CL],
                    in1=rc[:].to_broadcast([128, CL]), op=AL.mult)
                nc.vector.tensor_tensor(out=lg[:], in0=lg[:], in1=b2m[:],
                                        op=AL.add)
                mx = ep.tile([128, 1], f32, tag="mx")
                nc.vector.tensor_reduce(mx[:], lg[:], axis=ax_x, op=AL.max)
                nc.vector.tensor_tensor(
                    out=lgs[:, w * CL:(w + 1) * CL], in0=lg[:],
                    in1=mx[:].to_broadcast([128, CL]), op=AL.subtract)
                exs = ep.tile([128, CL], f32, tag="exs")
                nc.scalar.activation(exs[:], lgs[:, w * CL:(w + 1) * CL],
                                     AF.Exp, accum_out=sms[:, w:w + 1])

            # ---------------- batched log-softmax tail (one Ln table load)
            lnv = rp.tile([128, nwin], f32)
            nc.scalar.activation(lnv[:], sms[:], AF.Ln)
            for w in range(nwin):
                og = ep.tile([128, CL], f32, tag="og")
                nc.vector.tensor_tensor(
                    out=og[:], in0=lgs[:, w * CL:(w + 1) * CL],
                    in1=lnv[:, w:w + 1].to_broadcast([128, CL]),
                    op=AL.subtract)
                nc.sync.dma_start(out=out_d[w * 128:(w + 1) * 128, :],
                                  in_=og[:])

    nc.compile()
    return nc


def _host_inputs(inputs, cfg, percore):
    x = np.asarray(inputs["x"], np.float32)
    W1 = np.asarray(inputs["W1"], np.float32)
    a_s1 = np.asarray(inputs["a_src1"], np.float32)
    a_d1 = np.asarray(inputs["a_dst1"], np.float32)
    b1 = np.asarray(inputs["b1"], np.float32)
    W2 = np.asarray(inputs["W2"], np.float32)
    a_s2 = np.asarray(inputs["a_src2"], np.float32)
    a_d2 = np.asarray(inputs["a_dst2"], np.float32)
    b2 = np.asarray(inputs["b2"], np.float32)
    H, C1 = cfg["heads"], cfg["hid"]
    D1 = H * C1
    As = np.zeros((D1, H), np.float32)
    Ad = np.zeros((D1, H), np.float32)
    for hd in range(H):
        As[hd * C1:(hd + 1) * C1, hd] = a_s1[hd]
        Ad[hd * C1:(hd + 1) * C1, hd] = a_d1[hd]
    w1cat = np.concatenate([W1, W1 @ As, W1 @ Ad], axis=1)
    nsh, npad = cfg["nshard"], cfg["npad"]
    pr = nsh - (npad - 128)
    pmask = (np.arange(128) < pr).astype(np.float32)[:, None]
    pneg = (pmask - 1.0) * 1e30
    maps = []
    for c in range(cfg["ncores"]):
        xs = x[c * nsh:(c + 1) * nsh]
        xp = np.zeros((npad, cfg["f_in"]), np.float32)
        xp[:xs.shape[0]] = xs
        maps.append(dict(
            x_T=np.ascontiguousarray(xp.T), w1cat=w1cat,
            b1row=b1[None, :], a2srow=a_s2[0][None, :],
            a2drow=a_d2[0][None, :],
            w2b=W2.astype(BF), b2row=b2[None, :],
            idx_in=percore[c]["idx"], dcol_in=percore[c]["dcol"],
            drow_in=percore[c]["drow"], pmask=pmask, pneg=pneg,
        ))
    return maps


_CACHE = {}


def kernel(**inputs):
    from concourse import bass_utils

    cfg = FULL_CFG
    ei = np.asarray(inputs["edge_index"])
    src = ei[0].astype(np.int64)
    dst = ei[1].astype(np.int64)

    key = ("full", ei.shape[1])
    if key not in _CACHE:
        st, percore = prep_structure(src, dst, cfg)
        ncobj = build_nc(cfg, st)
        _CACHE[key] = (st, percore, ncobj)
    st, percore, ncobj = _CACHE[key]

    in_maps = _host_inputs(inputs, cfg, percore)
    res = bass_utils.run_bass_kernel_spmd(
        ncobj, in_maps, core_ids=list(range(cfg["ncores"])))
    outs = [res.results[c]["out"][:cfg["nshard"]]
            for c in range(cfg["ncores"])]
    return np.concatenate(outs, axis=0).astype(np.float32)


# revision 7
# speedup vs baseline: 1.0265x; 1.0265x over previous
"""GAT (2-layer, 8-head then 1-head) on 8 Trainium2 NeuronCores. v2.

Design: dst-shard nodes across 8 cores. Per layer, every core holds a bf16
node-feature table shard [NPAD, 128] (256B-pitch rows, only the leading
cols used: L1 row = [h(64)|al_hi(8)|al_lo(8)], L2 row = [y2(40)|hi|lo]),
AllGathered to all cores. Edges (dst-owned) are streamed in (block,
shard-PAIR, window)-aligned order with core-common structure; rows are
fetched with dma_gather using int16 idx into the 25088-row pair sub-table,
gathering only 160B (L1) / 96B (L2) of each 256B-pitch row. Per 128-edge
tile, one-hot S (edges x window-nodes, built via 4x tensor_scalar is_equal)
aggregates messages+exp on the PE into per-window PSUM; one-hot S^T looks
up al_dst per edge via PE. W2 is pre-applied in the L1 epilogue (linearity)
so L2 aggregates 40-wide. Softmax without max-subtraction (logits O(4));
log_softmax's Ln is batched at the end so only one act-table switch occurs.
"""
import sys
import numpy as np

sys.path.insert(0, "/opt/trn_rl_repo")
import ml_dtypes

BF = ml_dtypes.bfloat16

N = 100000
F_IN = 128
HID = 8
HEADS = 8
CLASSES = 40
NEG = 0.2
NC = 8

FULL_CFG = dict(
    ncores=8, nshard=12500, npad=12544, wb=4, ni_max=3072, f_in=128,
    heads=8, hid=8, classes=40, neg=0.2, nsp=4, ggc1=80, ggc2=48,
)

_PATCHED = [False]


def _patch_dma_gather():
    """Relax dma_gather's elem_size%256 assert (row PITCH stays 256B)."""
    if _PATCHED[0]:
        return
    import inspect
    import concourse.bass as bass

    src = inspect.getsource(bass.BassGpSimd.dma_gather)
    needle = "elem_size_bytes > 0 and elem_size_bytes % 256 == 0"
    assert needle in src, "dma_gather source changed; update patch"
    src = src.replace(needle, "elem_size_bytes > 0")
    src = "\n".join(line[4:] for line in src.split("\n"))
    ns = vars(bass).copy()
    exec(compile(src, "<patched_dma_gather>", "exec"), ns)
    bass.BassGpSimd.dma_gather = ns["dma_gather"]
    _PATCHED[0] = True


def _ceil(a, b):
    return -(-a // b)


# ---------------------------------------------------------------- host prep
def prep_structure(src, dst, cfg):
    """Build core-common call/tile/matmul structure + per-core arrays.

    Edges are grouped by (dst-core, block-of-4-windows, src shard-PAIR,
    window); group sizes are padded to the max over cores so the program is
    identical on all cores. Returns (st, percore).
    """
    nc_, nsh, npad, wb = cfg["ncores"], cfg["nshard"], cfg["npad"], cfg["wb"]
    nsp = cfg["nsp"]
    per_pair = 2 * nsh                     # 25000 global nodes per pair
    nwin = npad // 128
    nblk = _ceil(nwin, wb)
    NI = cfg["ni_max"]
    NT = NI // 128

    core = dst // nsh
    dstl = dst % nsh
    spair = src // per_pair
    # local row within the pair sub-table [2*npad rows]
    sl_in_pair = src % per_pair
    slocal = np.where(sl_in_pair < nsh, sl_in_pair,
                      sl_in_pair - nsh + npad)
    win = dstl // 128
    blk = win // wb

    nwb = [min(wb, nwin - b * wb) for b in range(nblk)]
    counts = np.zeros((nc_, nblk, nsp, nwin), dtype=np.int64)
    np.add.at(counts, (core, blk, spair, win), 1)
    common = counts.max(axis=0)            # [nblk, nsp, nwin]

    percore_edges = []
    for c in range(nc_):
        m = core == c
        key = (blk[m].astype(np.int64) * nsp + spair[m]) * nwin + win[m]
        o = np.argsort(key, kind="stable")
        percore_edges.append((key[o], slocal[m][o], dstl[m][o], win[m][o]))

    PAD_ROW = nsh          # pad row inside pair's first shard (pneg als)
    calls = []
    pc_idx = [[] for _ in range(nc_)]
    pc_dcol = [[] for _ in range(nc_)]
    pc_drow = [[] for _ in range(nc_)]
    win_mms = {}
    for b in range(nblk):
        for s in range(nsp):
            streams = []
            for c in range(nc_):
                kk, sl, dl, wn = percore_edges[c]
                segs = []
                for w in range(b * wb, b * wb + nwb[b]):
                    kval = (b * nsp + s) * nwin + w
                    lo = np.searchsorted(kk, kval, "left")
                    hi = np.searchsorted(kk, kval, "right")
                    n_common = common[b, s, w]
                    seg_s = np.full(n_common, PAD_ROW, dtype=np.int64)
                    seg_w = np.full(n_common, w, dtype=np.int64)
                    seg_s[: hi - lo] = sl[lo:hi]
                    seg_w[: hi - lo] = wn[lo:hi]
                    seg_d = np.full(n_common, -1, dtype=np.int64)
                    seg_d[: hi - lo] = dl[lo:hi]
                    segs.append(np.stack([seg_s, seg_w, seg_d]))
                streams.append(np.concatenate(segs, axis=1))
            L = streams[0].shape[1]
            pos = 0
            while pos < L:
                ni_real = min(NI, L - pos)
                ni = _ceil(ni_real, 128) * 128
                ntile = ni // 128
                wseg = streams[0][1][pos:pos + ni_real]
                tiles = []
                for t in range(ntile):
                    a, z = t * 128, min((t + 1) * 128, ni_real)
                    if a < ni_real:
                        tw = wseg[a:z]
                        w1 = int(tw.min())
                        wmax = int(tw.max())
                        assert wmax - w1 <= 1, "tile spans >2 windows"
                        straddle = wmax > w1
                    else:
                        w1, straddle = int(wseg[-1]), False
                    tiles.append((w1, straddle))
                cid = len(calls)
                mms = []
                for t, (w1, straddle) in enumerate(tiles):
                    ks = [0, 1] if straddle else [0]
                    for k in ks:
                        w = w1 + k
                        mm_id = (cid, t, k, w)
                        win_mms.setdefault((b, w), []).append(mm_id)
                        mms.append(mm_id)
                calls.append(dict(kind="stream", blk=b, shard=s, ni=ni,
                                  ntile=ntile, tiles=tiles, mms=mms))
                for c in range(nc_):
                    ss, ww, dd = streams[c]
                    sl_call = np.full(ni, PAD_ROW, dtype=np.int64)
                    rel_call = np.full(ni, 300.0, dtype=np.float64)
                    nreal = min(ni_real, L - pos)
                    sl_call[:nreal] = ss[pos:pos + nreal]
                    for t in range(ntile):
                        a, z = t * 128, min((t + 1) * 128, nreal)
                        if a >= nreal:
                            break
                        w1 = tiles[t][0]
                        dv = dd[a:z]
                        wv = ww[a:z]
                        rel = (wv - w1) * 128 + (dv - wv * 128)
                        rel = np.where(dv < 0, 300.0, rel)
                        rel_call[a:z] = rel
                    iw = sl_call.reshape(ni // 16, 16).T.astype(np.int16)
                    pc_idx[c].append(np.tile(iw, (8, 1)))
                    pc_dcol[c].append(
                        rel_call.reshape(ntile, 128).T.astype(np.float32))
                    pc_drow[c].append(rel_call.astype(BF))
                pos += ni_real
        for w in range(b * wb, b * wb + nwb[b]):
            cid = len(calls)
            mm_id = (cid, 0, 0, w)
            win_mms.setdefault((b, w), []).append(mm_id)
            calls.append(dict(kind="self", blk=b, w=w, mms=[mm_id]))

    startset, stopset = set(), set()
    for (b, w), ms in win_mms.items():
        startset.add(ms[0])
        stopset.add(ms[-1])
    for cl in calls:
        cl["flags"] = [(m, m in startset, m in stopset) for m in cl["mms"]]

    ncalls = len(calls)
    idx_t = [np.zeros((128, (NI // 16) * ncalls), np.int16) for _ in range(nc_)]
    dcol_t = [np.zeros((128, NT * ncalls), np.float32) for _ in range(nc_)]
    drow_t = [np.full((1, NI * ncalls), 300.0, BF) for _ in range(nc_)]
    for c in range(nc_):
        j = 0
        for i, cl in enumerate(calls):
            if cl["kind"] == "self":
                continue
            ni, nt = cl["ni"], cl["ntile"]
            idx_t[c][:, i * (NI // 16): i * (NI // 16) + ni // 16] = pc_idx[c][j]
            dcol_t[c][:, i * NT: i * NT + nt] = pc_dcol[c][j]
            drow_t[c][0, i * NI: i * NI + ni] = pc_drow[c][j]
            j += 1

    st = dict(calls=calls, nwin=nwin, nblk=nblk, nwb=nwb, ncalls=ncalls,
              win_mms=win_mms)
    percore = [dict(idx=idx_t[c], dcol=dcol_t[c], drow=drow_t[c])
               for c in range(nc_)]
    return st, percore


# ---------------------------------------------------------------- program
def build_nc(cfg, st):
    import concourse.bass as bass
    import concourse.bacc as bacc
    import concourse.tile as tile
    import concourse.mybir as mybir
    from concourse.masks import make_identity

    _patch_dma_gather()

    bf16, f32 = mybir.dt.bfloat16, mybir.dt.float32
    i16, i32 = mybir.dt.int16, mybir.dt.int32
    AL = mybir.AluOpType
    AF = mybir.ActivationFunctionType
    ax_x = mybir.AxisListType.X

    nc_, nsh, npad = cfg["ncores"], cfg["nshard"], cfg["npad"]
    nsp = cfg["nsp"]
    H, C1, CL = cfg["heads"], cfg["hid"], cfg["classes"]
    D1 = H * C1                      # 64
    NEGS = cfg["neg"]
    NI = cfg["ni_max"]
    NT = NI // 128
    nwin, nblk, nwb = st["nwin"], st["nblk"], st["nwb"]
    ncalls = st["ncalls"]
    NTOT = nc_ * npad
    ntile_x = npad // 128

    GC1 = D1 + 2 * H                 # 80: [h64 | hi8 | lo8]
    GC2 = CL + 2                     # 42: [y2 40 | hi | lo] (gather 48)
    GC2P = 48
    RH1 = D1 + H                     # 72
    RH2 = CL + 1                     # 41
    W1C = D1 + 2 * H                 # producer matmul width (80)

    nc = bacc.Bacc("TRN2", target_bir_lowering=False, debug=False,
                   enable_asserts=False, num_devices=nc_, num_swdge_queues=4)

    # ---- I/O
    x_T = nc.dram_tensor("x_T", [cfg["f_in"], npad], f32, kind="ExternalInput")
    w1cat = nc.dram_tensor("w1cat", [cfg["f_in"], W1C], f32,
                           kind="ExternalInput")
    b1row = nc.dram_tensor("b1row", [1, D1], f32, kind="ExternalInput")
    a2srow = nc.dram_tensor("a2srow", [1, CL], f32, kind="ExternalInput")
    a2drow = nc.dram_tensor("a2drow", [1, CL], f32, kind="ExternalInput")
    w2b = nc.dram_tensor("w2b", [D1, CL], bf16, kind="ExternalInput")
    b2row = nc.dram_tensor("b2row", [1, CL], f32, kind="ExternalInput")
    idx_in = nc.dram_tensor("idx_in", [128, (NI // 16) * ncalls], i16,
                            kind="ExternalInput")
    dcol_in = nc.dram_tensor("dcol_in", [128, NT * ncalls], f32,
                             kind="ExternalInput")
    drow_in = nc.dram_tensor("drow_in", [1, NI * ncalls], bf16,
                             kind="ExternalInput")
    pmask_in = nc.dram_tensor("pmask", [128, 1], f32, kind="ExternalInput")
    pneg_in = nc.dram_tensor("pneg", [128, 1], f32, kind="ExternalInput")
    out_d = nc.dram_tensor("out", [npad, CL], f32, kind="ExternalOutput")

    with tile.TileContext(nc) as tc:
        with (
            tc.tile_pool(name="const", bufs=1) as cpool,
            tc.tile_pool(name="sb", bufs=3) as sb,
            tc.tile_pool(name="gpool", bufs=4) as gp,
            tc.tile_pool(name="spool", bufs=3) as sp,
            tc.tile_pool(name="meta", bufs=4) as mp,
            tc.tile_pool(name="epi", bufs=2) as ep,
            tc.tile_pool(name="res", bufs=1) as rp,
            tc.tile_pool(name="pwin", bufs=max(nwb) + 1, space="PSUM") as pw,
            tc.tile_pool(name="pald", bufs=1, space="PSUM") as pa,
            tc.tile_pool(name="pma", bufs=1, space="PSUM") as pm,
            tc.tile_pool(name="pmb", bufs=1, space="PSUM") as pmb,
            tc.tile_pool(name="dram", bufs=1, space="DRAM") as dp,
        ):
            # ---------- constants
            ident = cpool.tile([128, 128], f32)
            make_identity(nc, ident[:])
            identb = cpool.tile([128, 128], bf16)
            nc.vector.tensor_copy(identb[:], ident[:])
            iota_i = cpool.tile([128, 128], i32)
            nc.gpsimd.iota(iota_i[:], pattern=[[1, 128]], base=0,
                           channel_multiplier=0)
            iota_mat = cpool.tile([128, 128], bf16)
            nc.vector.tensor_copy(iota_mat[:], iota_i[:])
            iota_mat2 = cpool.tile([128, 128], bf16)
            nc.vector.tensor_scalar_add(iota_mat2[:], iota_mat[:], 128.0)
            ic_i = cpool.tile([128, 1], i32)
            nc.gpsimd.iota(ic_i[:], pattern=[[0, 1]], base=0,
                           channel_multiplier=1)
            iota_col = cpool.tile([128, 1], f32)
            nc.vector.tensor_copy(iota_col[:], ic_i[:])
            iota_col2 = cpool.tile([128, 1], f32)
            nc.vector.tensor_scalar_add(iota_col2[:], iota_col[:], 128.0)
            b1m = cpool.tile([128, D1], f32)
            nc.sync.dma_start(out=b1m[:], in_=b1row[:].to_broadcast([128, D1]))
            a2sm = cpool.tile([128, CL], f32)
            nc.sync.dma_start(out=a2sm[:], in_=a2srow[:].to_broadcast([128, CL]))
            a2dm = cpool.tile([128, CL], f32)
            nc.sync.dma_start(out=a2dm[:], in_=a2drow[:].to_broadcast([128, CL]))
            b2m = cpool.tile([128, CL], f32)
            nc.sync.dma_start(out=b2m[:], in_=b2row[:].to_broadcast([128, CL]))
            w1c_sb = cpool.tile([cfg["f_in"], W1C], f32)
            nc.sync.dma_start(out=w1c_sb[:], in_=w1cat[:])
            w2b_sb = cpool.tile([D1, CL], bf16)
            nc.sync.dma_start(out=w2b_sb[:], in_=w2b[:])
            pmask = cpool.tile([128, 1], f32)
            nc.sync.dma_start(out=pmask[:], in_=pmask_in[:])
            pneg = cpool.tile([128, 1], f32)
            nc.sync.dma_start(out=pneg[:], in_=pneg_in[:])
            zcol = cpool.tile([128, 1], f32)
            nc.vector.memset(zcol[:], 0.0)

            # resident tables
            al1w = rp.tile([128, 2 * H * nwin], bf16)     # [hi8|lo8] per win
            al2w = rp.tile([128, 2 * nwin], bf16)         # [hi|lo] per win
            lgs = rp.tile([128, CL * nwin], f32)          # logits - max
            sms = rp.tile([128, nwin], f32)               # sum(exp)

            # DRAM tables (256B-pitch rows; only leading cols used)
            t1_own = dp.tile([npad, 128], bf16)
            t1_full = dp.tile([NTOT, 128], bf16)
            t2_own = dp.tile([npad, 128], bf16)
            t2_full = dp.tile([NTOT, 128], bf16)

            # ---------------- P0: produce T1 + al1 window tables
            for t in range(ntile_x):
                xt = sb.tile([cfg["f_in"], 128], f32, tag="xt")
                nc.sync.dma_start(out=xt[:], in_=x_T[:, t * 128:(t + 1) * 128])
                ps = pm.tile([128, W1C], f32, space="PSUM", tag="pm")
                nc.tensor.matmul(ps[:], lhsT=xt[:], rhs=w1c_sb[:],
                                 start=True, stop=True)
                t1sb = sb.tile([128, GC1], bf16, tag="t1sb")
                nc.vector.tensor_copy(t1sb[:, 0:D1], ps[:, 0:D1])
                nc.vector.tensor_copy(t1sb[:, D1:D1 + H], ps[:, D1:D1 + H])
                nc.vector.tensor_tensor(out=t1sb[:, D1 + H:D1 + 2 * H],
                                        in0=ps[:, D1:D1 + H],
                                        in1=t1sb[:, D1:D1 + H],
                                        op=AL.subtract)
                o = 2 * H * t
                nc.vector.tensor_copy(al1w[:, o:o + H], ps[:, D1 + H:W1C])
                nc.vector.tensor_tensor(out=al1w[:, o + H:o + 2 * H],
                                        in0=ps[:, D1 + H:W1C],
                                        in1=al1w[:, o:o + H], op=AL.subtract)
                if t == ntile_x - 1 and npad > nsh:
                    nc.vector.scalar_tensor_tensor(
                        out=t1sb[:, 0:D1], in0=t1sb[:, 0:D1], scalar=pmask[:],
                        in1=zcol[:].to_broadcast([128, D1]),
                        op0=AL.mult, op1=AL.add)
                    nc.vector.scalar_tensor_tensor(
                        out=t1sb[:, D1:D1 + H], in0=t1sb[:, D1:D1 + H],
                        scalar=pmask[:], in1=pneg[:].to_broadcast([128, H]),
                        op0=AL.mult, op1=AL.add)
                    nc.vector.scalar_tensor_tensor(
                        out=t1sb[:, D1 + H:D1 + 2 * H],
                        in0=t1sb[:, D1 + H:D1 + 2 * H],
                        scalar=pmask[:], in1=zcol[:].to_broadcast([128, H]),
                        op0=AL.mult, op1=AL.add)
                nc.sync.dma_start(out=t1_own[t * 128:(t + 1) * 128, 0:GC1],
                                  in_=t1sb[:])

            nc.gpsimd.collective_compute(
                "AllGather", AL.bypass,
                replica_groups=[list(range(nc_))],
                ins=[t1_own.opt()], outs=[t1_full.opt()],
            )

            # ---------------- shared edge pass
            def edge_pass(tfull, town, alw, nal, mc, gc, gcp, rhw):
                """nal: attn scalars/edge; mc: msg cols; gc: used row cols;
                gcp: gathered cols; rhw: rhs width = mc + nal."""
                blk_psums = {}
                cph = mc // nal
                for ci, cl in enumerate(st["calls"]):
                    if cl["kind"] == "self":
                        w = cl["w"]
                        gs = gp.tile([128, gcp], bf16, tag="gs")
                        nc.sync.dma_start(
                            out=gs[:],
                            in_=town[w * 128:(w + 1) * 128, 0:gcp])
                        es = sb.tile([128, nal], f32, tag="es")
                        nc.vector.tensor_tensor(
                            out=es[:], in0=gs[:, mc:mc + nal],
                            in1=gs[:, mc + nal:mc + 2 * nal], op=AL.add)
                        ed = sb.tile([128, nal], f32, tag="ed")
                        nc.vector.tensor_tensor(
                            out=ed[:], in0=alw[:, 2 * nal * w:2 * nal * w + nal],
                            in1=alw[:, 2 * nal * w + nal:2 * nal * (w + 1)],
                            op=AL.add)
                        nc.vector.tensor_tensor(out=es[:], in0=es[:], in1=ed[:],
                                                op=AL.add)
                        nc.vector.scalar_tensor_tensor(
                            out=es[:], in0=es[:], scalar=NEGS, in1=es[:],
                            op0=AL.mult, op1=AL.max)
                        rhs_s = sb.tile([128, RH1], bf16, tag="rhss")
                        nc.scalar.activation(rhs_s[:, mc:mc + nal], es[:],
                                             AF.Exp)
                        nc.vector.tensor_tensor(
                            out=rhs_s[:, 0:mc].rearrange("p (a c) -> p a c",
                                                         c=cph),
                            in0=gs[:, 0:mc].rearrange("p (a c) -> p a c",
                                                      c=cph),
                            in1=rhs_s[:, mc:mc + nal]
                            .broadcast_to([128, nal, cph]),
                            op=AL.mult)
                        (mm, fstart, fstop) = cl["flags"][0]
                        key = (cl["blk"], w)
                        pt = blk_psums.get(key)
                        if pt is None:
                            pt = pw.tile([128, RH1], f32, space="PSUM",
                                         tag="pwin")
                            blk_psums[key] = pt
                        nc.tensor.matmul(pt[:, 0:rhw], lhsT=identb[:],
                                         rhs=rhs_s[:, 0:rhw],
                                         start=fstart, stop=fstop)
                        if fstop:
                            yield w, blk_psums.pop(key)
                        continue
                    b, s_, ni, nt = cl["blk"], cl["shard"], cl["ni"], cl["ntile"]
                    dcol = mp.tile([128, NT], f32, tag="dcol")
                    nc.sync.dma_start(out=dcol[:, 0:nt],
                                      in_=dcol_in[:, ci * NT:ci * NT + nt])
                    drep = mp.tile([128, NI], bf16, tag="drep")
                    nc.sync.dma_start(
                        out=drep[:, 0:ni],
                        in_=drow_in[:, ci * NI:ci * NI + ni]
                        .to_broadcast([128, ni]))
                    idxt = mp.tile([128, NI // 16], i16, tag="idxt")
                    nc.sync.dma_start(
                        out=idxt[:, 0:ni // 16],
                        in_=idx_in[:, ci * (NI // 16):ci * (NI // 16) + ni // 16])
                    # gather (gcp cols of each 256B-pitch row); the SWDGE
                    # ucode caps num_idxs at 1024, so issue sub-gathers on
                    # rotating queues
                    g = gp.tile([128, NT * gcp], bf16, tag="g")
                    qn = [0]
                    for a in range(0, ni, 1024):
                        z = min(a + 1024, ni)
                        nc.gpsimd.dma_gather(
                            g[:, (a // 128) * gcp:(z // 128) * gcp]
                            .rearrange("p (b e) -> p b e", e=gcp),
                            tfull[s_ * 2 * npad:(s_ + 1) * 2 * npad, 0:gcp],
                            idxt[:, a // 16:z // 16], z - a, z - a, gcp,
                            elem_step=128, single_packet=True,
                            queue_num=(ci + qn[0]) % 4)
                        qn[0] += 1
                    # one-hot S builds (4x tensor_scalar)
                    s1 = sp.tile([128, NI], bf16, tag="s1")
                    any_straddle = any(x[1] for x in cl["tiles"])
                    if any_straddle:
                        s2 = sp.tile([128, NI], bf16, tag="s2")
                    for t, (w1, straddle) in enumerate(cl["tiles"]):
                        nc.vector.tensor_scalar(
                            out=s1[:, t * 128:(t + 1) * 128],
                            in0=iota_mat[:], scalar1=dcol[:, t:t + 1],
                            scalar2=None, op0=AL.is_equal)
                        if straddle:
                            nc.vector.tensor_scalar(
                                out=s2[:, t * 128:(t + 1) * 128],
                                in0=iota_mat2[:], scalar1=dcol[:, t:t + 1],
                                scalar2=None, op0=AL.is_equal)
                    st1 = sp.tile([128, NI], bf16, tag="st1")
                    nc.vector.tensor_scalar(
                        out=st1[:, 0:ni], in0=drep[:, 0:ni],
                        scalar1=iota_col[:], scalar2=None, op0=AL.is_equal)
                    if any_straddle:
                        st2 = sp.tile([128, NI], bf16, tag="st2")
                        nc.vector.tensor_scalar(
                            out=st2[:, 0:ni], in0=drep[:, 0:ni],
                            scalar1=iota_col2[:], scalar2=None,
                            op0=AL.is_equal)
                    # al_dst lookup per tile -> psum [128, nt*nal];
                    # hi and lo parts accumulate on the PE (start/stop chain)
                    pald = pa.tile([128, NT * nal], f32, space="PSUM",
                                   tag="pald")
                    for t, (w1, straddle) in enumerate(cl["tiles"]):
                        ks = [0, 1] if straddle else [0]
                        mmparts = [(k, part) for k in ks for part in (0, 1)]
                        for i, (k, part) in enumerate(mmparts):
                            w = w1 + k
                            stm = st1 if k == 0 else st2
                            o = 2 * nal * w + part * nal
                            nc.tensor.matmul(
                                pald[:, t * nal:(t + 1) * nal],
                                lhsT=stm[:, t * 128:(t + 1) * 128],
                                rhs=alw[:, o:o + nal],
                                start=(i == 0), stop=(i == len(mmparts) - 1))
                    # e = (als_hi+als_lo) + (ald_hi+ald_lo); leaky
                    eals = sb.tile([128, NT * nal], f32, tag="eals")
                    nc.vector.tensor_tensor(
                        out=eals[:, 0:nt * nal]
                        .rearrange("p (b a) -> p b a", a=nal),
                        in0=g[:, 0:nt * gcp].rearrange("p (b e) -> p b e",
                                                       e=gcp)
                        [:, :, mc:mc + nal],
                        in1=g[:, 0:nt * gcp].rearrange("p (b e) -> p b e",
                                                       e=gcp)
                        [:, :, mc + nal:mc + 2 * nal],
                        op=AL.add)
                    ee = sb.tile([128, NT * nal], f32, tag="ee")
                    nc.vector.tensor_tensor(out=ee[:, 0:nt * nal],
                                            in0=eals[:, 0:nt * nal],
                                            in1=pald[:, 0:nt * nal], op=AL.add)
                    nc.vector.scalar_tensor_tensor(
                        out=ee[:, 0:nt * nal], in0=ee[:, 0:nt * nal],
                        scalar=NEGS, in1=ee[:, 0:nt * nal],
                        op0=AL.mult, op1=AL.max)
                    # rhs assembly
                    rhs = sb.tile([128, NT * rhw], bf16, tag="rhs")
                    nc.scalar.activation(
                        rhs[:, 0:nt * rhw].rearrange("p (b r) -> p b r", r=rhw)
                        [:, :, mc:mc + nal],
                        ee[:, 0:nt * nal].rearrange("p (b a) -> p b a", a=nal),
                        AF.Exp)
                    nc.vector.tensor_tensor(
                        out=rhs[:, 0:nt * rhw]
                        .rearrange("p (b r) -> p b r", r=rhw)[:, :, 0:mc]
                        .rearrange("p b (a c) -> p b a c", c=cph),
                        in0=g[:, 0:nt * gcp].rearrange("p (b e) -> p b e",
                                                       e=gcp)
                        [:, :, 0:mc].rearrange("p b (a c) -> p b a c", c=cph),
                        in1=rhs[:, 0:nt * rhw]
                        .rearrange("p (b r) -> p b r", r=rhw)
                        [:, :, mc:mc + nal]
                        .broadcast_to([128, nt, nal, cph]),
                        op=AL.mult)
                    # aggregation matmuls
                    for (mm, fstart, fstop) in cl["flags"]:
                        _, t, k, w = mm
                        smat = s1 if k == 0 else s2
                        key = (cl["blk"], w)
                        pt = blk_psums.get(key)
                        if pt is None:
                            pt = pw.tile([128, RH1], f32, space="PSUM",
                                         tag="pwin")
                            blk_psums[key] = pt
                        nc.tensor.matmul(
                            pt[:, 0:rhw],
                            lhsT=smat[:, t * 128:(t + 1) * 128],
                            rhs=rhs[:, t * rhw:(t + 1) * rhw],
                            start=fstart, stop=fstop)
                    for (mm, fstart, fstop) in cl["flags"]:
                        if not fstop:
                            continue
                        _, t, k, w = mm
                        key = (cl["blk"], w)
                        yield w, blk_psums.pop(key)

            # ---------------- L1 pass + epilogue -> T2 (W2 pre-applied)
            for w, pt in edge_pass(t1_full, t1_own, al1w, H, D1, GC1,
                                   cfg["ggc1"], RH1):
                rc = ep.tile([128, H], f32, tag="rc1")
                nc.vector.reciprocal(rc[:], pt[:, D1:D1 + H])
                nc.vector.tensor_scalar_min(rc[:], rc[:], 1e30)
                o1 = ep.tile([128, D1], f32, tag="o1")
                nc.vector.tensor_tensor(
                    out=o1[:].rearrange("p (h c) -> p h c", c=C1),
                    in0=pt[:, 0:D1].rearrange("p (h c) -> p h c", c=C1),
                    in1=rc[:].broadcast_to([128, H, C1]),
                    op=AL.mult)
                nc.vector.tensor_tensor(out=o1[:], in0=o1[:], in1=b1m[:],
                                        op=AL.add)
                r1 = ep.tile([128, D1], f32, tag="r1")
                nc.scalar.activation(r1[:], o1[:], AF.Relu)
                # y2 = relu1 @ W2 via transpose + matmul
                trp = pmb.tile([D1, 128], f32, space="PSUM", tag="trp")
                nc.tensor.transpose(out=trp[:], in_=r1[:], identity=ident[:])
                trs = ep.tile([D1, 128], bf16, tag="trs")
                nc.vector.tensor_copy(trs[:], trp[:])
                y2p = pm.tile([128, CL], f32, space="PSUM", tag="pm")
                nc.tensor.matmul(y2p[:], lhsT=trs[:], rhs=w2b_sb[:],
                                 start=True, stop=True)
                t2sb = ep.tile([128, GC2P], bf16, tag="t2sb")
                nc.vector.tensor_copy(t2sb[:, 0:CL], y2p[:])
                tmp = ep.tile([128, CL], f32, tag="altmp")
                a2s = ep.tile([128, 1], f32, tag="a2s")
                nc.vector.tensor_tensor(out=tmp[:], in0=y2p[:], in1=a2sm[:],
                                        op=AL.mult)
                nc.vector.tensor_reduce(a2s[:], tmp[:], axis=ax_x, op=AL.add)
                a2d = ep.tile([128, 1], f32, tag="a2d")
                nc.vector.tensor_tensor(out=tmp[:], in0=y2p[:], in1=a2dm[:],
                                        op=AL.mult)
                nc.vector.tensor_reduce(a2d[:], tmp[:], axis=ax_x, op=AL.add)
                nc.vector.tensor_copy(t2sb[:, CL:CL + 1], a2s[:])
                nc.vector.tensor_tensor(out=t2sb[:, CL + 1:CL + 2],
                                        in0=a2s[:], in1=t2sb[:, CL:CL + 1],
                                        op=AL.subtract)
                nc.vector.memset(t2sb[:, CL + 2:GC2P], 0.0)
                nc.vector.tensor_copy(al2w[:, 2 * w:2 * w + 1], a2d[:])
                nc.vector.tensor_tensor(out=al2w[:, 2 * w + 1:2 * w + 2],
                                        in0=a2d[:], in1=al2w[:, 2 * w:2 * w + 1],
                                        op=AL.subtract)
                if w == nwin - 1 and npad > nsh:
                    nc.vector.scalar_tensor_tensor(
                        out=t2sb[:, 0:CL], in0=t2sb[:, 0:CL], scalar=pmask[:],
                        in1=zcol[:].to_broadcast([128, CL]),
                        op0=AL.mult, op1=AL.add)
                    nc.vector.scalar_tensor_tensor(
                        out=t2sb[:, CL:CL + 1], in0=t2sb[:, CL:CL + 1],
                        scalar=pmask[:], in1=pneg[:], op0=AL.mult, op1=AL.add)
                    nc.vector.scalar_tensor_tensor(
                        out=t2sb[:, CL + 1:CL + 2], in0=t2sb[:, CL + 1:CL + 2],
                        scalar=pmask[:], in1=zcol[:], op0=AL.mult, op1=AL.add)
                    nc.vector.scalar_tensor_tensor(
                        out=al2w[:, 2 * w:2 * w + 2],
                        in0=al2w[:, 2 * w:2 * w + 2],
                        scalar=pmask[:], in1=zcol[:].to_broadcast([128, 2]),
                        op0=AL.mult, op1=AL.add)
                nc.sync.dma_start(out=t2_own[w * 128:(w + 1) * 128, 0:GC2P],
                                  in_=t2sb[:])

            nc.gpsimd.collective_compute(
                "AllGather", AL.bypass,
                replica_groups=[list(range(nc_))],
                ins=[t2_own.opt()], outs=[t2_full.opt()],
            )

            # ---------------- L2 pass + epilogue -> resident logits
            for w, pt in edge_pass(t2_full, t2_own, al2w, 1, CL, GC2,
                                   cfg["ggc2"], RH2):
                rc = ep.tile([128, 1], f32, tag="rc2")
                nc.vector.reciprocal(rc[:], pt[:, CL:CL + 1])
                nc.vector.tensor_scalar_min(rc[:], rc[:], 1e30)
                lg = ep.tile([128, CL], f32, tag="lg")
                nc.vector.tensor_tensor(
                    out=lg[:], in0=pt[:, 0:CL],
                    in1=rc[:].to_broadcast([128, CL]), op=AL.mult)
                nc.vector.tensor_tensor(out=lg[:], in0=lg[:], in1=b2m[:],
                                        op=AL.add)
                mx = ep.tile([128, 1], f32, tag="mx")
                nc.vector.tensor_reduce(mx[:], lg[:], axis=ax_x, op=AL.max)
                nc.vector.tensor_tensor(
                    out=lgs[:, w * CL:(w + 1) * CL], in0=lg[:],
                    in1=mx[:].to_broadcast([128, CL]), op=AL.subtract)
                exs = ep.tile([128, CL], f32, tag="exs")
                nc.scalar.activation(exs[:], lgs[:, w * CL:(w + 1) * CL],
                                     AF.Exp, accum_out=sms[:, w:w + 1])

            # ---------------- batched log-softmax tail (one Ln table load)
            lnv = rp.tile([128, nwin], f32)
            nc.scalar.activation(lnv[:], sms[:], AF.Ln)
            for w in range(nwin):
                og = ep.tile([128, CL], f32, tag="og")
                nc.vector.tensor_tensor(
                    out=og[:], in0=lgs[:, w * CL:(w + 1) * CL],
                    in1=lnv[:, w:w + 1].to_broadcast([128, CL]),
                    op=AL.subtract)
                nc.sync.dma_start(out=out_d[w * 128:(w + 1) * 128, :],
                                  in_=og[:])

    nc.compile()
    return nc


def _host_inputs(inputs, cfg, percore):
    x = np.asarray(inputs["x"], np.float32)
    W1 = np.asarray(inputs["W1"], np.float32)
    a_s1 = np.asarray(inputs["a_src1"], np.float32)
    a_d1 = np.asarray(inputs["a_dst1"], np.float32)
    b1 = np.asarray(inputs["b1"], np.float32)
    W2 = np.asarray(inputs["W2"], np.float32)
    a_s2 = np.asarray(inputs["a_src2"], np.float32)
    a_d2 = np.asarray(inputs["a_dst2"], np.float32)
    b2 = np.asarray(inputs["b2"], np.float32)
    H, C1 = cfg["heads"], cfg["hid"]
    D1 = H * C1
    As = np.zeros((D1, H), np.float32)
    Ad = np.zeros((D1, H), np.float32)
    for hd in range(H):
        As[hd * C1:(hd + 1) * C1, hd] = a_s1[hd]
        Ad[hd * C1:(hd + 1) * C1, hd] = a_d1[hd]
    w1cat = np.concatenate([W1, W1 @ As, W1 @ Ad], axis=1)
    nsh, npad = cfg["nshard"], cfg["npad"]
    pr = nsh - (npad - 128)
    pmask = (np.arange(128) < pr).astype(np.float32)[:, None]
    pneg = (pmask - 1.0) * 1e30
    maps = []
    for c in range(cfg["ncores"]):
        xs = x[c * nsh:(c + 1) * nsh]
        xp = np.zeros((npad, cfg["f_in"]), np.float32)
        xp[:xs.shape[0]] = xs
        maps.append(dict(
            x_T=np.ascontiguousarray(xp.T), w1cat=w1cat,
            b1row=b1[None, :], a2srow=a_s2[0][None, :],
            a2drow=a_d2[0][None, :],
            w2b=W2.astype(BF), b2row=b2[None, :],
            idx_in=percore[c]["idx"], dcol_in=percore[c]["dcol"],
            drow_in=percore[c]["drow"], pmask=pmask, pneg=pneg,
        ))
    return maps


_CACHE = {}


def kernel(**inputs):
    from concourse import bass_utils

    cfg = FULL_CFG
    ei = np.asarray(inputs["edge_index"])
    src = ei[0].astype(np.int64)
    dst = ei[1].astype(np.int64)

    key = ("full", ei.shape[1])
    if key not in _CACHE:
        st, percore = prep_structure(src, dst, cfg)
        ncobj = build_nc(cfg, st)
        _CACHE[key] = (st, percore, ncobj)
    st, percore, ncobj = _CACHE[key]

    in_maps = _host_inputs(inputs, cfg, percore)
    res = bass_utils.run_bass_kernel_spmd(
        ncobj, in_maps, core_ids=list(range(cfg["ncores"])))
    outs = [res.results[c]["out"][:cfg["nshard"]]
            for c in range(cfg["ncores"])]
    return np.concatenate(outs, axis=0).astype(np.float32)


# revision 11
# speedup vs baseline: 1.0685x; 1.0410x over previous
"""GAT (2-layer, 8-head then 1-head) on 8 Trainium2 NeuronCores. v2.

Design: dst-shard nodes across 8 cores. Per layer, every core holds a bf16
node-feature table shard [NPAD, 128] (256B-pitch rows, only the leading
cols used: L1 row = [h(64)|al_hi(8)|al_lo(8)], L2 row = [y2(40)|hi|lo]),
AllGathered to all cores. Edges (dst-owned) are streamed in (block,
shard-PAIR, window)-aligned order with core-common structure; rows are
fetched with dma_gather using int16 idx into the 25088-row pair sub-table,
gathering only 160B (L1) / 96B (L2) of each 256B-pitch row. Per 128-edge
tile, one-hot S (edges x window-nodes, built via 4x tensor_scalar is_equal)
aggregates messages+exp on the PE into per-window PSUM; one-hot S^T looks
up al_dst per edge via PE. W2 is pre-applied in the L1 epilogue (linearity)
so L2 aggregates 40-wide. Softmax without max-subtraction (logits O(4));
log_softmax's Ln is batched at the end so only one act-table switch occurs.
"""
import sys
import numpy as np

sys.path.insert(0, "/opt/trn_rl_repo")
import ml_dtypes

BF = ml_dtypes.bfloat16

N = 100000
F_IN = 128
HID = 8
HEADS = 8
CLASSES = 40
NEG = 0.2
NC = 8

FULL_CFG = dict(
    ncores=8, nshard=12500, npad=12544, wb=4, ni_max=3072, f_in=128,
    heads=8, hid=8, classes=40, neg=0.2, nsp=4, ggc1=80, ggc2=48,
)

_PATCHED = [False]


def _patch_dma_gather():
    """Relax dma_gather's elem_size%256 assert (row PITCH stays 256B)."""
    if _PATCHED[0]:
        return
    import inspect
    import concourse.bass as bass

    src = inspect.getsource(bass.BassGpSimd.dma_gather)
    needle = "elem_size_bytes > 0 and elem_size_bytes % 256 == 0"
    assert needle in src, "dma_gather source changed; update patch"
    src = src.replace(needle, "elem_size_bytes > 0")
    src = "\n".join(line[4:] for line in src.split("\n"))
    ns = vars(bass).copy()
    exec(compile(src, "<patched_dma_gather>", "exec"), ns)
    bass.BassGpSimd.dma_gather = ns["dma_gather"]
    _PATCHED[0] = True


def _ceil(a, b):
    return -(-a // b)


# ---------------------------------------------------------------- host prep
def prep_structure(src, dst, cfg):
    """Build core-common call/tile/matmul structure + per-core arrays.

    Edges are grouped by (dst-core, block-of-4-windows, src shard-PAIR,
    window); group sizes are padded to the max over cores so the program is
    identical on all cores. Returns (st, percore).
    """
    nc_, nsh, npad, wb = cfg["ncores"], cfg["nshard"], cfg["npad"], cfg["wb"]
    nsp = cfg["nsp"]
    per_pair = 2 * nsh                     # 25000 global nodes per pair
    nwin = npad // 128
    nblk = _ceil(nwin, wb)
    NI = cfg["ni_max"]
    NT = NI // 128

    core = dst // nsh
    dstl = dst % nsh
    spair = src // per_pair
    # local row within the pair sub-table [2*npad rows]
    sl_in_pair = src % per_pair
    slocal = np.where(sl_in_pair < nsh, sl_in_pair,
                      sl_in_pair - nsh + npad)
    win = dstl // 128
    blk = win // wb

    nwb = [min(wb, nwin - b * wb) for b in range(nblk)]
    counts = np.zeros((nc_, nblk, nsp, nwin), dtype=np.int64)
    np.add.at(counts, (core, blk, spair, win), 1)
    common = counts.max(axis=0)            # [nblk, nsp, nwin]

    percore_edges = []
    for c in range(nc_):
        m = core == c
        key = (blk[m].astype(np.int64) * nsp + spair[m]) * nwin + win[m]
        o = np.argsort(key, kind="stable")
        percore_edges.append((key[o], slocal[m][o], dstl[m][o], win[m][o]))

    PAD_ROW = nsh          # pad row inside pair's first shard (pneg als)
    calls = []
    pc_idx = [[] for _ in range(nc_)]
    pc_idx2 = [[] for _ in range(nc_)]
    pc_dcol = [[] for _ in range(nc_)]
    pc_drow = [[] for _ in range(nc_)]
    win_mms = {}
    for b in range(nblk):
        for s in range(nsp):
            streams = []
            for c in range(nc_):
                kk, sl, dl, wn = percore_edges[c]
                segs = []
                for w in range(b * wb, b * wb + nwb[b]):
                    kval = (b * nsp + s) * nwin + w
                    lo = np.searchsorted(kk, kval, "left")
                    hi = np.searchsorted(kk, kval, "right")
                    n_common = common[b, s, w]
                    seg_s = np.full(n_common, PAD_ROW, dtype=np.int64)
                    seg_w = np.full(n_common, w, dtype=np.int64)
                    seg_s[: hi - lo] = sl[lo:hi]
                    seg_w[: hi - lo] = wn[lo:hi]
                    seg_d = np.full(n_common, -1, dtype=np.int64)
                    seg_d[: hi - lo] = dl[lo:hi]
                    segs.append(np.stack([seg_s, seg_w, seg_d]))
                streams.append(np.concatenate(segs, axis=1))
            L = streams[0].shape[1]
            pos = 0
            while pos < L:
                ni_real = min(NI, L - pos)
                ni = _ceil(ni_real, 128) * 128
                ntile = ni // 128
                wseg = streams[0][1][pos:pos + ni_real]
                tiles = []
                for t in range(ntile):
                    a, z = t * 128, min((t + 1) * 128, ni_real)
                    if a < ni_real:
                        tw = wseg[a:z]
                        w1 = int(tw.min())
                        wmax = int(tw.max())
                        assert wmax - w1 <= 1, "tile spans >2 windows"
                        straddle = wmax > w1
                    else:
                        w1, straddle = int(wseg[-1]), False
                    tiles.append((w1, straddle))
                cid = len(calls)
                mms = []
                for t, (w1, straddle) in enumerate(tiles):
                    ks = [0, 1] if straddle else [0]
                    for k in ks:
                        w = w1 + k
                        mm_id = (cid, t, k, w)
                        win_mms.setdefault((b, w), []).append(mm_id)
                        mms.append(mm_id)
                calls.append(dict(kind="stream", blk=b, shard=s, ni=ni,
                                  ntile=ntile, tiles=tiles, mms=mms))
                for c in range(nc_):
                    ss, ww, dd = streams[c]
                    sl_call = np.full(ni, PAD_ROW, dtype=np.int64)
                    dl_call = np.zeros(ni, dtype=np.int64)
                    rel_call = np.full(ni, 300.0, dtype=np.float64)
                    nreal = min(ni_real, L - pos)
                    sl_call[:nreal] = ss[pos:pos + nreal]
                    dseg = dd[pos:pos + nreal]
                    dl_call[:nreal] = np.where(dseg < 0, 0, dseg)
                    for t in range(ntile):
                        a, z = t * 128, min((t + 1) * 128, nreal)
                        if a >= nreal:
                            break
                        w1 = tiles[t][0]
                        dv = dd[a:z]
                        wv = ww[a:z]
                        rel = (wv - w1) * 128 + (dv - wv * 128)
                        rel = np.where(dv < 0, 300.0, rel)
                        rel_call[a:z] = rel
                    iw = sl_call.reshape(ni // 16, 16).T.astype(np.int16)
                    pc_idx[c].append(np.tile(iw, (8, 1)))
                    iw2 = dl_call.reshape(ni // 16, 16).T.astype(np.int16)
                    pc_idx2[c].append(np.tile(iw2, (8, 1)))
                    pc_dcol[c].append(
                        rel_call.reshape(ntile, 128).T.astype(BF))
                    pc_drow[c].append(rel_call.astype(BF))
                pos += ni_real
        for w in range(b * wb, b * wb + nwb[b]):
            cid = len(calls)
            mm_id = (cid, 0, 0, w)
            win_mms.setdefault((b, w), []).append(mm_id)
            calls.append(dict(kind="self", blk=b, w=w, mms=[mm_id]))

    startset, stopset = set(), set()
    for (b, w), ms in win_mms.items():
        startset.add(ms[0])
        stopset.add(ms[-1])
    for cl in calls:
        cl["flags"] = [(m, m in startset, m in stopset) for m in cl["mms"]]

    ncalls = len(calls)
    idx_t = [np.zeros((128, (NI // 16) * ncalls), np.int16) for _ in range(nc_)]
    idx2_t = [np.zeros((128, (NI // 16) * ncalls), np.int16) for _ in range(nc_)]
    dcol_t = [np.zeros((128, NT * ncalls), BF) for _ in range(nc_)]
    drow_t = [np.full((1, NI * ncalls), 300.0, BF) for _ in range(nc_)]
    for c in range(nc_):
        j = 0
        for i, cl in enumerate(calls):
            if cl["kind"] == "self":
                continue
            ni, nt = cl["ni"], cl["ntile"]
            idx_t[c][:, i * (NI // 16): i * (NI // 16) + ni // 16] = pc_idx[c][j]
            idx2_t[c][:, i * (NI // 16): i * (NI // 16) + ni // 16] = pc_idx2[c][j]
            dcol_t[c][:, i * NT: i * NT + nt] = pc_dcol[c][j]
            drow_t[c][0, i * NI: i * NI + ni] = pc_drow[c][j]
            j += 1

    st = dict(calls=calls, nwin=nwin, nblk=nblk, nwb=nwb, ncalls=ncalls,
              win_mms=win_mms)
    percore = [dict(idx=idx_t[c], idx2=idx2_t[c], dcol=dcol_t[c],
                    drow=drow_t[c])
               for c in range(nc_)]
    return st, percore


# ---------------------------------------------------------------- program
def build_nc(cfg, st):
    import concourse.bass as bass
    import concourse.bacc as bacc
    import concourse.tile as tile
    import concourse.mybir as mybir
    from concourse.masks import make_identity

    _patch_dma_gather()

    bf16, f32 = mybir.dt.bfloat16, mybir.dt.float32
    i16, i32 = mybir.dt.int16, mybir.dt.int32
    AL = mybir.AluOpType
    AF = mybir.ActivationFunctionType
    ax_x = mybir.AxisListType.X

    nc_, nsh, npad = cfg["ncores"], cfg["nshard"], cfg["npad"]
    nsp = cfg["nsp"]
    H, C1, CL = cfg["heads"], cfg["hid"], cfg["classes"]
    D1 = H * C1                      # 64
    NEGS = cfg["neg"]
    NI = cfg["ni_max"]
    NT = NI // 128
    nwin, nblk, nwb = st["nwin"], st["nblk"], st["nwb"]
    ncalls = st["ncalls"]
    NTOT = nc_ * npad
    ntile_x = npad // 128

    GC1 = D1 + 2 * H                 # 80: [h64 | hi8 | lo8]
    GC2 = CL + 2                     # 42: [y2 40 | hi | lo] (gather 48)
    GC2P = 48
    RH1 = D1 + H                     # 72
    RH2 = CL + 1                     # 41
    W1C = D1 + 2 * H                 # producer matmul width (80)

    nc = bacc.Bacc("TRN2", target_bir_lowering=False, debug=False,
                   enable_asserts=False, num_devices=nc_, num_swdge_queues=4)

    # ---- I/O
    x_T = nc.dram_tensor("x_T", [cfg["f_in"], npad], f32, kind="ExternalInput")
    w1cat = nc.dram_tensor("w1cat", [cfg["f_in"], W1C], f32,
                           kind="ExternalInput")
    b1row = nc.dram_tensor("b1row", [1, D1], f32, kind="ExternalInput")
    a2srow = nc.dram_tensor("a2srow", [1, CL], f32, kind="ExternalInput")
    a2drow = nc.dram_tensor("a2drow", [1, CL], f32, kind="ExternalInput")
    w2b = nc.dram_tensor("w2b", [D1, CL], bf16, kind="ExternalInput")
    b2row = nc.dram_tensor("b2row", [1, CL], f32, kind="ExternalInput")
    idx_in = nc.dram_tensor("idx_in", [128, (NI // 16) * ncalls], i16,
                            kind="ExternalInput")
    idx2_in = nc.dram_tensor("idx2_in", [128, (NI // 16) * ncalls], i16,
                             kind="ExternalInput")
    dcol_in = nc.dram_tensor("dcol_in", [128, NT * ncalls], bf16,
                             kind="ExternalInput")
    drow_in = nc.dram_tensor("drow_in", [1, NI * ncalls], bf16,
                             kind="ExternalInput")
    pmask_in = nc.dram_tensor("pmask", [128, 1], f32, kind="ExternalInput")
    pneg_in = nc.dram_tensor("pneg", [128, 1], f32, kind="ExternalInput")
    out_d = nc.dram_tensor("out", [npad, CL], f32, kind="ExternalOutput")

    with tile.TileContext(nc) as tc:
        with (
            tc.tile_pool(name="const", bufs=1) as cpool,
            tc.tile_pool(name="sb", bufs=3) as sb,
            tc.tile_pool(name="gpool", bufs=4) as gp,
            tc.tile_pool(name="spool", bufs=3) as sp,
            tc.tile_pool(name="meta", bufs=4) as mp,
            tc.tile_pool(name="epi", bufs=2) as ep,
            tc.tile_pool(name="res", bufs=1) as rp,
            tc.tile_pool(name="pwin", bufs=max(nwb) + 1, space="PSUM") as pw,
            tc.tile_pool(name="pald", bufs=1, space="PSUM") as pa,
            tc.tile_pool(name="pma", bufs=1, space="PSUM") as pm,
            tc.tile_pool(name="pmb", bufs=1, space="PSUM") as pmb,
            tc.tile_pool(name="dram", bufs=1, space="DRAM") as dp,
        ):
            # ---------- constants
            ident = cpool.tile([128, 128], f32)
            make_identity(nc, ident[:])
            identb = cpool.tile([128, 128], bf16)
            nc.vector.tensor_copy(identb[:], ident[:])
            iota_i = cpool.tile([128, 128], i32)
            nc.gpsimd.iota(iota_i[:], pattern=[[1, 128]], base=0,
                           channel_multiplier=0)
            iota_mat = cpool.tile([128, 128], bf16)
            nc.vector.tensor_copy(iota_mat[:], iota_i[:])
            iota_mat2 = cpool.tile([128, 128], bf16)
            nc.vector.tensor_scalar_add(iota_mat2[:], iota_mat[:], 128.0)
            ic_i = cpool.tile([128, 1], i32)
            nc.gpsimd.iota(ic_i[:], pattern=[[0, 1]], base=0,
                           channel_multiplier=1)
            iota_col = cpool.tile([128, 1], f32)
            nc.vector.tensor_copy(iota_col[:], ic_i[:])
            iota_col2 = cpool.tile([128, 1], f32)
            nc.vector.tensor_scalar_add(iota_col2[:], iota_col[:], 128.0)
            b1m = cpool.tile([128, D1], f32)
            nc.sync.dma_start(out=b1m[:], in_=b1row[:].to_broadcast([128, D1]))
            a2sm = cpool.tile([128, CL], f32)
            nc.sync.dma_start(out=a2sm[:], in_=a2srow[:].to_broadcast([128, CL]))
            a2dm = cpool.tile([128, CL], f32)
            nc.sync.dma_start(out=a2dm[:], in_=a2drow[:].to_broadcast([128, CL]))
            b2m = cpool.tile([128, CL], f32)
            nc.sync.dma_start(out=b2m[:], in_=b2row[:].to_broadcast([128, CL]))
            w1c_sb = cpool.tile([cfg["f_in"], W1C], f32)
            nc.sync.dma_start(out=w1c_sb[:], in_=w1cat[:])
            w2b_sb = cpool.tile([D1, CL], bf16)
            nc.sync.dma_start(out=w2b_sb[:], in_=w2b[:])
            pmask = cpool.tile([128, 1], f32)
            nc.sync.dma_start(out=pmask[:], in_=pmask_in[:])
            pneg = cpool.tile([128, 1], f32)
            nc.sync.dma_start(out=pneg[:], in_=pneg_in[:])
            zcol = cpool.tile([128, 1], f32)
            nc.vector.memset(zcol[:], 0.0)

            # resident tables
            al1w = rp.tile([128, 2 * H * nwin], bf16)     # [hi8|lo8] per win
            al2w = rp.tile([128, 2 * nwin], bf16)         # [hi|lo] per win
            lgs = rp.tile([128, CL * nwin], f32)          # logits - max
            sms = rp.tile([128, nwin], f32)               # sum(exp)

            # DRAM tables (256B-pitch rows; only leading cols used)
            t1_own = dp.tile([npad, 128], bf16)
            t1_full = dp.tile([NTOT, 128], bf16)
            t2_own = dp.tile([npad, 128], bf16)
            t2_full = dp.tile([NTOT, 128], bf16)

            # ---------------- P0: produce T1 + al1 window tables
            for t in range(ntile_x):
                xt = sb.tile([cfg["f_in"], 128], f32, tag="xt")
                nc.sync.dma_start(out=xt[:], in_=x_T[:, t * 128:(t + 1) * 128])
                ps = pm.tile([128, W1C], f32, space="PSUM", tag="pm")
                nc.tensor.matmul(ps[:], lhsT=xt[:], rhs=w1c_sb[:],
                                 start=True, stop=True)
                t1sb = sb.tile([128, GC1], bf16, tag="t1sb")
                nc.vector.tensor_copy(t1sb[:, 0:D1], ps[:, 0:D1])
                nc.vector.tensor_copy(t1sb[:, D1:D1 + H], ps[:, D1:D1 + H])
                nc.vector.tensor_tensor(out=t1sb[:, D1 + H:D1 + 2 * H],
                                        in0=ps[:, D1:D1 + H],
                                        in1=t1sb[:, D1:D1 + H],
                                        op=AL.subtract)
                o = 2 * H * t
                nc.vector.tensor_copy(al1w[:, o:o + H], ps[:, D1 + H:W1C])
                nc.vector.tensor_tensor(out=al1w[:, o + H:o + 2 * H],
                                        in0=ps[:, D1 + H:W1C],
                                        in1=al1w[:, o:o + H], op=AL.subtract)
                if t == ntile_x - 1 and npad > nsh:
                    nc.vector.scalar_tensor_tensor(
                        out=t1sb[:, 0:D1], in0=t1sb[:, 0:D1], scalar=pmask[:],
                        in1=zcol[:].to_broadcast([128, D1]),
                        op0=AL.mult, op1=AL.add)
                    nc.vector.scalar_tensor_tensor(
                        out=t1sb[:, D1:D1 + H], in0=t1sb[:, D1:D1 + H],
                        scalar=pmask[:], in1=pneg[:].to_broadcast([128, H]),
                        op0=AL.mult, op1=AL.add)
                    nc.vector.scalar_tensor_tensor(
                        out=t1sb[:, D1 + H:D1 + 2 * H],
                        in0=t1sb[:, D1 + H:D1 + 2 * H],
                        scalar=pmask[:], in1=zcol[:].to_broadcast([128, H]),
                        op0=AL.mult, op1=AL.add)
                nc.sync.dma_start(out=t1_own[t * 128:(t + 1) * 128, 0:GC1],
                                  in_=t1sb[:])

            nc.gpsimd.collective_compute(
                "AllGather", AL.bypass,
                replica_groups=[list(range(nc_))],
                ins=[t1_own.opt()], outs=[t1_full.opt()],
            )

            # ---------------- shared edge pass
            swq = [0]

            def edge_pass(tfull, town, alw, nal, mc, gc, gcp, rhw):
                """nal: attn scalars/edge; mc: msg cols; gc: used row cols;
                gcp: gathered cols; rhw: rhs width = mc + nal."""
                blk_psums = {}
                cph = mc // nal
                for ci, cl in enumerate(st["calls"]):
                    if cl["kind"] == "self":
                        w = cl["w"]
                        gs = gp.tile([128, gcp], bf16, tag="gs")
                        nc.sync.dma_start(
                            out=gs[:],
                            in_=town[w * 128:(w + 1) * 128, 0:gcp])
                        es = sb.tile([128, nal], f32, tag="es")
                        nc.vector.tensor_tensor(
                            out=es[:], in0=gs[:, mc:mc + nal],
                            in1=gs[:, mc + nal:mc + 2 * nal], op=AL.add)
                        ed = sb.tile([128, nal], f32, tag="ed")
                        nc.vector.tensor_tensor(
                            out=ed[:], in0=alw[:, 2 * nal * w:2 * nal * w + nal],
                            in1=alw[:, 2 * nal * w + nal:2 * nal * (w + 1)],
                            op=AL.add)
                        nc.vector.tensor_tensor(out=es[:], in0=es[:], in1=ed[:],
                                                op=AL.add)
                        nc.vector.scalar_tensor_tensor(
                            out=es[:], in0=es[:], scalar=NEGS, in1=es[:],
                            op0=AL.mult, op1=AL.max)
                        rhs_s = sb.tile([128, RH1], bf16, tag="rhss")
                        nc.scalar.activation(rhs_s[:, mc:mc + nal], es[:],
                                             AF.Exp)
                        nc.vector.tensor_tensor(
                            out=rhs_s[:, 0:mc].rearrange("p (a c) -> p a c",
                                                         c=cph),
                            in0=gs[:, 0:mc].rearrange("p (a c) -> p a c",
                                                      c=cph),
                            in1=rhs_s[:, mc:mc + nal]
                            .broadcast_to([128, nal, cph]),
                            op=AL.mult)
                        (mm, fstart, fstop) = cl["flags"][0]
                        key = (cl["blk"], w)
                        pt = blk_psums.get(key)
                        if pt is None:
                            pt = pw.tile([128, RH1], f32, space="PSUM",
                                         tag="pwin")
                            blk_psums[key] = pt
                        nc.tensor.matmul(pt[:, 0:rhw], lhsT=identb[:],
                                         rhs=rhs_s[:, 0:rhw],
                                         start=fstart, stop=fstop)
                        if fstop:
                            yield w, blk_psums.pop(key)
                        continue
                    b, s_, ni, nt = cl["blk"], cl["shard"], cl["ni"], cl["ntile"]
                    dcol = mp.tile([128, NT], bf16, tag="dcol")
                    nc.sync.dma_start(out=dcol[:, 0:nt],
                                      in_=dcol_in[:, ci * NT:ci * NT + nt])
                    idxt = mp.tile([128, NI // 16], i16, tag="idxt")
                    nc.sync.dma_start(
                        out=idxt[:, 0:ni // 16],
                        in_=idx_in[:, ci * (NI // 16):ci * (NI // 16) + ni // 16])
                    idx2t = mp.tile([128, NI // 16], i16, tag="idx2t")
                    nc.sync.dma_start(
                        out=idx2t[:, 0:ni // 16],
                        in_=idx2_in[:, ci * (NI // 16):ci * (NI // 16) + ni // 16])
                    # gather (gcp cols of each 256B-pitch row); the SWDGE
                    # ucode caps num_idxs at 1024, so issue sub-gathers on
                    # rotating queues
                    g = gp.tile([128, NT * gcp], bf16, tag="g")
                    for a in range(0, ni, 1024):
                        z = min(a + 1024, ni)
                        nc.gpsimd.dma_gather(
                            g[:, (a // 128) * gcp:(z // 128) * gcp]
                            .rearrange("p (b e) -> p b e", e=gcp),
                            tfull[s_ * 2 * npad:(s_ + 1) * 2 * npad, 0:gcp],
                            idxt[:, a // 16:z // 16], z - a, z - a, gcp,
                            elem_step=128, single_packet=True,
                            queue_num=swq[0] % 4)
                        swq[0] += 1
                    # one-hot S build (single fat is_equal per call)
                    im1 = iota_mat[:].rearrange("p (o n) -> p o n", o=1)
                    im2 = iota_mat2[:].rearrange("p (o n) -> p o n", o=1)
                    s1 = sp.tile([128, NI], bf16, tag="s1")
                    nc.vector.tensor_tensor(
                        out=s1[:, 0:ni].rearrange("p (b n) -> p b n", n=128),
                        in0=dcol[:, 0:nt].broadcast_to([128, nt, 128]),
                        in1=im1.broadcast_to([128, nt, 128]),
                        op=AL.is_equal)
                    any_straddle = any(x[1] for x in cl["tiles"])
                    if any_straddle:
                        s2 = sp.tile([128, NI], bf16, tag="s2")
                        nc.vector.tensor_tensor(
                            out=s2[:, 0:ni].rearrange("p (b n) -> p b n",
                                                      n=128),
                            in0=dcol[:, 0:nt].broadcast_to([128, nt, 128]),
                            in1=im2.broadcast_to([128, nt, 128]),
                            op=AL.is_equal)
                    # al_dst per edge via second gather from own table
                    # (dst-local rows, al cols only)
                    g2 = gp.tile([128, NT * 2 * nal], bf16, tag="g2")
                    for a in range(0, ni, 1024):
                        z = min(a + 1024, ni)
                        nc.gpsimd.dma_gather(
                            g2[:, (a // 128) * 2 * nal:(z // 128) * 2 * nal]
                            .rearrange("p (b e) -> p b e", e=2 * nal),
                            town[:, mc:mc + 2 * nal],
                            idx2t[:, a // 16:z // 16], z - a, z - a, 2 * nal,
                            elem_step=128, single_packet=True,
                            queue_num=swq[0] % 4)
                        swq[0] += 1
                    # e = (als_hi+als_lo) + (ald_hi+ald_lo); leaky
                    eals = sb.tile([128, NT * nal], f32, tag="eals")
                    nc.vector.tensor_tensor(
                        out=eals[:, 0:nt * nal]
                        .rearrange("p (b a) -> p b a", a=nal),
                        in0=g[:, 0:nt * gcp].rearrange("p (b e) -> p b e",
                                                       e=gcp)
                        [:, :, mc:mc + nal],
                        in1=g[:, 0:nt * gcp].rearrange("p (b e) -> p b e",
                                                       e=gcp)
                        [:, :, mc + nal:mc + 2 * nal],
                        op=AL.add)
                    eald = sb.tile([128, NT * nal], f32, tag="eald")
                    nc.vector.tensor_tensor(
                        out=eald[:, 0:nt * nal]
                        .rearrange("p (b a) -> p b a", a=nal),
                        in0=g2[:, 0:nt * 2 * nal]
                        .rearrange("p (b e) -> p b e", e=2 * nal)[:, :, 0:nal],
                        in1=g2[:, 0:nt * 2 * nal]
                        .rearrange("p (b e) -> p b e", e=2 * nal)
                        [:, :, nal:2 * nal],
                        op=AL.add)
                    ee = sb.tile([128, NT * nal], f32, tag="ee")
                    nc.vector.tensor_tensor(out=ee[:, 0:nt * nal],
                                            in0=eals[:, 0:nt * nal],
                                            in1=eald[:, 0:nt * nal], op=AL.add)
                    nc.vector.scalar_tensor_tensor(
                        out=ee[:, 0:nt * nal], in0=ee[:, 0:nt * nal],
                        scalar=NEGS, in1=ee[:, 0:nt * nal],
                        op0=AL.mult, op1=AL.max)
                    # rhs assembly
                    rhs = sb.tile([128, NT * rhw], bf16, tag="rhs")
                    nc.scalar.activation(
                        rhs[:, 0:nt * rhw].rearrange("p (b r) -> p b r", r=rhw)
                        [:, :, mc:mc + nal],
                        ee[:, 0:nt * nal].rearrange("p (b a) -> p b a", a=nal),
                        AF.Exp)
                    nc.vector.tensor_tensor(
                        out=rhs[:, 0:nt * rhw]
                        .rearrange("p (b r) -> p b r", r=rhw)[:, :, 0:mc]
                        .rearrange("p b (a c) -> p b a c", c=cph),
                        in0=g[:, 0:nt * gcp].rearrange("p (b e) -> p b e",
                                                       e=gcp)
                        [:, :, 0:mc].rearrange("p b (a c) -> p b a c", c=cph),
                        in1=rhs[:, 0:nt * rhw]
                        .rearrange("p (b r) -> p b r", r=rhw)
                        [:, :, mc:mc + nal]
                        .broadcast_to([128, nt, nal, cph]),
                        op=AL.mult)
                    # aggregation matmuls
                    for (mm, fstart, fstop) in cl["flags"]:
                        _, t, k, w = mm
                        smat = s1 if k == 0 else s2
                        key = (cl["blk"], w)
                        pt = blk_psums.get(key)
                        if pt is None:
                            pt = pw.tile([128, RH1], f32, space="PSUM",
                                         tag="pwin")
                            blk_psums[key] = pt
                        nc.tensor.matmul(
                            pt[:, 0:rhw],
                            lhsT=smat[:, t * 128:(t + 1) * 128],
                            rhs=rhs[:, t * rhw:(t + 1) * rhw],
                            start=fstart, stop=fstop)
                    for (mm, fstart, fstop) in cl["flags"]:
                        if not fstop:
                            continue
                        _, t, k, w = mm
                        key = (cl["blk"], w)
                        yield w, blk_psums.pop(key)

            # ---------------- L1 pass + epilogue -> T2 (W2 pre-applied)
            for w, pt in edge_pass(t1_full, t1_own, al1w, H, D1, GC1,
                                   cfg["ggc1"], RH1):
                rc = ep.tile([128, H], f32, tag="rc1")
                nc.vector.reciprocal(rc[:], pt[:, D1:D1 + H])
                nc.vector.tensor_scalar_min(rc[:], rc[:], 1e30)
                o1 = ep.tile([128, D1], f32, tag="o1")
                nc.vector.tensor_tensor(
                    out=o1[:].rearrange("p (h c) -> p h c", c=C1),
                    in0=pt[:, 0:D1].rearrange("p (h c) -> p h c", c=C1),
                    in1=rc[:].broadcast_to([128, H, C1]),
                    op=AL.mult)
                nc.vector.tensor_tensor(out=o1[:], in0=o1[:], in1=b1m[:],
                                        op=AL.add)
                r1 = ep.tile([128, D1], f32, tag="r1")
                nc.scalar.activation(r1[:], o1[:], AF.Relu)
                # y2 = relu1 @ W2 via transpose + matmul
                trp = pmb.tile([D1, 128], f32, space="PSUM", tag="trp")
                nc.tensor.transpose(out=trp[:], in_=r1[:], identity=ident[:])
                trs = ep.tile([D1, 128], bf16, tag="trs")
                nc.vector.tensor_copy(trs[:], trp[:])
                y2p = pm.tile([128, CL], f32, space="PSUM", tag="pm")
                nc.tensor.matmul(y2p[:], lhsT=trs[:], rhs=w2b_sb[:],
                                 start=True, stop=True)
                t2sb = ep.tile([128, GC2P], bf16, tag="t2sb")
                nc.vector.tensor_copy(t2sb[:, 0:CL], y2p[:])
                tmp = ep.tile([128, CL], f32, tag="altmp")
                a2s = ep.tile([128, 1], f32, tag="a2s")
                nc.vector.tensor_tensor(out=tmp[:], in0=y2p[:], in1=a2sm[:],
                                        op=AL.mult)
                nc.vector.tensor_reduce(a2s[:], tmp[:], axis=ax_x, op=AL.add)
                a2d = ep.tile([128, 1], f32, tag="a2d")
                nc.vector.tensor_tensor(out=tmp[:], in0=y2p[:], in1=a2dm[:],
                                        op=AL.mult)
                nc.vector.tensor_reduce(a2d[:], tmp[:], axis=ax_x, op=AL.add)
                nc.vector.tensor_copy(t2sb[:, CL:CL + 1], a2s[:])
                nc.vector.tensor_tensor(out=t2sb[:, CL + 1:CL + 2],
                                        in0=a2s[:], in1=t2sb[:, CL:CL + 1],
                                        op=AL.subtract)
                nc.vector.memset(t2sb[:, CL + 2:GC2P], 0.0)
                nc.vector.tensor_copy(al2w[:, 2 * w:2 * w + 1], a2d[:])
                nc.vector.tensor_tensor(out=al2w[:, 2 * w + 1:2 * w + 2],
                                        in0=a2d[:], in1=al2w[:, 2 * w:2 * w + 1],
                                        op=AL.subtract)
                if w == nwin - 1 and npad > nsh:
                    nc.vector.scalar_tensor_tensor(
                        out=t2sb[:, 0:CL], in0=t2sb[:, 0:CL], scalar=pmask[:],
                        in1=zcol[:].to_broadcast([128, CL]),
                        op0=AL.mult, op1=AL.add)
                    nc.vector.scalar_tensor_tensor(
                        out=t2sb[:, CL:CL + 1], in0=t2sb[:, CL:CL + 1],
                        scalar=pmask[:], in1=pneg[:], op0=AL.mult, op1=AL.add)
                    nc.vector.scalar_tensor_tensor(
                        out=t2sb[:, CL + 1:CL + 2], in0=t2sb[:, CL + 1:CL + 2],
                        scalar=pmask[:], in1=zcol[:], op0=AL.mult, op1=AL.add)
                    nc.vector.scalar_tensor_tensor(
                        out=al2w[:, 2 * w:2 * w + 2],
                        in0=al2w[:, 2 * w:2 * w + 2],
                        scalar=pmask[:], in1=zcol[:].to_broadcast([128, 2]),
                        op0=AL.mult, op1=AL.add)
                nc.sync.dma_start(out=t2_own[w * 128:(w + 1) * 128, 0:GC2P],
                                  in_=t2sb[:])

            nc.gpsimd.collective_compute(
                "AllGather", AL.bypass,
                replica_groups=[list(range(nc_))],
                ins=[t2_own.opt()], outs=[t2_full.opt()],
            )

            # ---------------- L2 pass + epilogue -> resident logits
            for w, pt in edge_pass(t2_full, t2_own, al2w, 1, CL, GC2,
                                   cfg["ggc2"], RH2):
                rc = ep.tile([128, 1], f32, tag="rc2")
                nc.vector.reciprocal(rc[:], pt[:, CL:CL + 1])
                nc.vector.tensor_scalar_min(rc[:], rc[:], 1e30)
                lg = ep.tile([128, CL], f32, tag="lg")
                nc.vector.tensor_tensor(
                    out=lg[:], in0=pt[:, 0:CL],
                    in1=rc[:].to_broadcast([128, CL]), op=AL.mult)
                nc.vector.tensor_tensor(out=lg[:], in0=lg[:], in1=b2m[:],
                                        op=AL.add)
                mx = ep.tile([128, 1], f32, tag="mx")
                nc.vector.tensor_reduce(mx[:], lg[:], axis=ax_x, op=AL.max)
                nc.vector.tensor_tensor(
                    out=lgs[:, w * CL:(w + 1) * CL], in0=lg[:],
                    in1=mx[:].to_broadcast([128, CL]), op=AL.subtract)
                exs = ep.tile([128, CL], f32, tag="exs")
                nc.scalar.activation(exs[:], lgs[:, w * CL:(w + 1) * CL],
                                     AF.Exp, accum_out=sms[:, w:w + 1])

            # ---------------- batched log-softmax tail (one Ln table load)
            lnv = rp.tile([128, nwin], f32)
            nc.scalar.activation(lnv[:], sms[:], AF.Ln)
            for w in range(nwin):
                og = ep.tile([128, CL], f32, tag="og")
                nc.vector.tensor_tensor(
                    out=og[:], in0=lgs[:, w * CL:(w + 1) * CL],
                    in1=lnv[:, w:w + 1].to_broadcast([128, CL]),
                    op=AL.subtract)
                nc.sync.dma_start(out=out_d[w * 128:(w + 1) * 128, :],
                                  in_=og[:])

    nc.compile()

    # tile_sem_assignment assigns DMASW sem lanes round-robin over Pool-DMA
    # instructions in SCHEDULED order, which may differ from emission order.
    # The SWDGE queue must match the sem lane (the sim/hw lock each sem to
    # one queue), so fix queue_num up from the recorded lane.
    from concourse.tile_sem_assignment import PROC_NAME_TO_IDX
    lane_of = {v: int(k[5:]) for k, v in PROC_NAME_TO_IDX.items()
               if k.startswith("DMASW")}
    nq = 4
    for blk in nc.main_func.blocks:
        for inst in blk.instructions:
            if isinstance(inst, mybir.InstDMAGatherAnt):
                proc = getattr(inst, "bass_scheduled_proc", None)
                if proc in lane_of:
                    inst.queue_num = lane_of[proc] % nq
    return nc


def _host_inputs(inputs, cfg, percore):
    x = np.asarray(inputs["x"], np.float32)
    W1 = np.asarray(inputs["W1"], np.float32)
    a_s1 = np.asarray(inputs["a_src1"], np.float32)
    a_d1 = np.asarray(inputs["a_dst1"], np.float32)
    b1 = np.asarray(inputs["b1"], np.float32)
    W2 = np.asarray(inputs["W2"], np.float32)
    a_s2 = np.asarray(inputs["a_src2"], np.float32)
    a_d2 = np.asarray(inputs["a_dst2"], np.float32)
    b2 = np.asarray(inputs["b2"], np.float32)
    H, C1 = cfg["heads"], cfg["hid"]
    D1 = H * C1
    As = np.zeros((D1, H), np.float32)
    Ad = np.zeros((D1, H), np.float32)
    for hd in range(H):
        As[hd * C1:(hd + 1) * C1, hd] = a_s1[hd]
        Ad[hd * C1:(hd + 1) * C1, hd] = a_d1[hd]
    w1cat = np.concatenate([W1, W1 @ As, W1 @ Ad], axis=1)
    nsh, npad = cfg["nshard"], cfg["npad"]
    pr = nsh - (npad - 128)
    pmask = (np.arange(128) < pr).astype(np.float32)[:, None]
    pneg = (pmask - 1.0) * 1e30
    maps = []
    for c in range(cfg["ncores"]):
        xs = x[c * nsh:(c + 1) * nsh]
        xp = np.zeros((npad, cfg["f_in"]), np.float32)
        xp[:xs.shape[0]] = xs
        maps.append(dict(
            x_T=np.ascontiguousarray(xp.T), w1cat=w1cat,
            b1row=b1[None, :], a2srow=a_s2[0][None, :],
            a2drow=a_d2[0][None, :],
            w2b=W2.astype(BF), b2row=b2[None, :],
            idx_in=percore[c]["idx"], idx2_in=percore[c]["idx2"],
            dcol_in=percore[c]["dcol"],
            drow_in=percore[c]["drow"], pmask=pmask, pneg=pneg,
        ))
    return maps


_CACHE = {}


def kernel(**inputs):
    from concourse import bass_utils

    cfg = FULL_CFG
    ei = np.asarray(inputs["edge_index"])
    src = ei[0].astype(np.int64)
    dst = ei[1].astype(np.int64)

    key = ("full", ei.shape[1])
    if key not in _CACHE:
        st, percore = prep_structure(src, dst, cfg)
        ncobj = build_nc(cfg, st)
        _CACHE[key] = (st, percore, ncobj)
    st, percore, ncobj = _CACHE[key]

    in_maps = _host_inputs(inputs, cfg, percore)
    res = bass_utils.run_bass_kernel_spmd(
        ncobj, in_maps, core_ids=list(range(cfg["ncores"])))
    outs = [res.results[c]["out"][:cfg["nshard"]]
            for c in range(cfg["ncores"])]
    return np.concatenate(outs, axis=0).astype(np.float32)
